# revision 1
# baseline (speedup 1.0000x reference)
"""Trainium2 Bass kernel: MultiHeadAttention with QK-RMSNorm + partial rotary,
causal softmax. B=4, T=2048, D=1024, H=16, HD=64, fp32.

Sharding: 8 cores = 4 batches x 2 head-groups (8 heads each). Each core:
  - QKV projections for its batch, restricted to its 512 head-dims
  - causal attention for its 8 heads
  - partial output projection (its 512 contraction dims, all 1024 outputs)
Host sums the two head-group partials per batch (the all-reduce) and
transposes back.

Layout: fully transposed pipeline, zero on-chip transposes:
  xt [D, T] -> Qt/Kt [hd, t] (proj with wT as lhsT), V [t, hd]
  scores St[j, i] = Kt^T-row . Qt-col  (transposed scores, causal over j<=i)
  softmax without max-subtraction (RMS-normed q,k bound |s| <= 8)
  AV: lhsT = [V | ones] (j, 65), rhs = exp(St) -> Ot [65, i] with
      row 64 = softmax denominator (free)
  out-proj: lhsT = woT chunk, rhs = normalized Ot -> yt [dout, t]
Matmuls run as float32r (full-rate fp32 on the PE at N>=256).
"""

import numpy as np
from contextlib import ExitStack

import concourse.bass as bass
import concourse.tile as tile
import concourse.mybir as mybir
from concourse import bacc

F32 = mybir.dt.float32
MM_DT = mybir.dt.float32r # float32r = full-rate; float32 = exact, 1/4-rate
AF = mybir.ActivationFunctionType
MULT = mybir.AluOpType.mult
ADD = mybir.AluOpType.add

D = 1024   # model dim
DH = 512   # head-group width per core (8 heads x 64)
NH = 8     # heads per core
HD = 64    # head dim
NKC = D // 128   # k-chunks over model dim
EPS = 1e-6


def _r(ap):
    return ap.bitcast(MM_DT)


def _cast_dve(nc, ap):
    """In-place round f32 -> f32r (DVE) so walrus accepts it as mm input."""
    nc.vector.tensor_copy(ap.bitcast(MM_DT), ap)


def _cast_act(nc, ap):
    nc.scalar.copy(ap.bitcast(MM_DT), ap)


def build_kernel(nc: bass.Bass, T: int = 2048, dbg: bool = False):
    """Trace the per-core program. T parameterized for fast sim smoke tests."""
    NTT = T // 512     # 512-wide t/i blocks
    NTS = T // 128     # 128-wide t/j chunks

    if dbg:
        dqt = nc.dram_tensor("dqt", [4, 128, T], F32, kind="ExternalOutput").ap()
        dkt = nc.dram_tensor("dkt", [4, 128, T], F32, kind="ExternalOutput").ap()
        dqr = nc.dram_tensor("dqr", [4, 128, T], F32, kind="ExternalOutput").ap()
        dv = nc.dram_tensor("dv", [NTS, 128, NH * 65], F32,
                            kind="ExternalOutput").ap()
        dp = nc.dram_tensor("dp", [2, 128, T], F32, kind="ExternalOutput").ap()
        dot = nc.dram_tensor("dot", [4, 128, T], F32, kind="ExternalOutput").ap()

    xt = nc.dram_tensor("xt", [D, T], F32, kind="ExternalInput").ap()
    wqt = nc.dram_tensor("wqt", [D, DH], F32, kind="ExternalInput").ap()
    wkt = nc.dram_tensor("wkt", [D, DH], F32, kind="ExternalInput").ap()
    wvt = nc.dram_tensor("wvt", [D, DH], F32, kind="ExternalInput").ap()
    wot = nc.dram_tensor("wot", [DH, D], F32, kind="ExternalInput").ap()
    c2d = nc.dram_tensor("c2", [128, T], F32, kind="ExternalInput").ap()
    s2d = nc.dram_tensor("s2", [128, T], F32, kind="ExternalInput").ap()
    pswapd = nc.dram_tensor("pswap", [128, 128], F32, kind="ExternalInput").ap()
    bdiagd = nc.dram_tensor("bdiag", [128, 128], F32, kind="ExternalInput").ap()
    trid = nc.dram_tensor("trimask", [128, 2048], F32, kind="ExternalInput").ap()
    yt = nc.dram_tensor("yt", [D, T], F32, kind="ExternalOutput").ap()

    with tile.TileContext(nc) as tc, ExitStack() as ctx:
        # ---- persistent pools -------------------------------------------
        qk_pool = ctx.enter_context(tc.tile_pool(name="qk", bufs=1))
        v_pool = ctx.enter_context(tc.tile_pool(name="v", bufs=1))
        const_pool = ctx.enter_context(tc.tile_pool(name="const", bufs=1))

        # Qt/Kt: [128, T] tiles, partition = head-dim (2 heads per tile)
        qt_s = [qk_pool.tile([128, T], F32, name=f"qt{j}") for j in range(4)]
        kt_s = [qk_pool.tile([128, T], F32, name=f"kt{j}") for j in range(4)]
        # V (+ones col): [128, 8*65] per 128-token chunk
        v_s = [v_pool.tile([128, NH * 65], F32, name=f"vt{j}") for j in range(NTS)]
        pswap = const_pool.tile([128, 128], F32, name="pswap_s")
        bdiag = const_pool.tile([128, 128], F32, name="bdiag_s")
        nc.sync.dma_start(_r(pswap[:]), _r(pswapd[:]))
        nc.sync.dma_start(_r(bdiag[:]), _r(bdiagd[:]))
        epsb = const_pool.tile([128, 1], F32, name="epsb")
        nc.gpsimd.memset(epsb[:], 8.0 * EPS)
        onesc = const_pool.tile([128, NH], F32, name="onesc")
        nc.gpsimd.memset(onesc[:], 1.0)
        ones64 = const_pool.tile([128, 1], F32, name="ones64")
        nc.vector.tensor_copy(_r(ones64[:]), onesc[:, 0:1])
        # nk columns: rsqrt(8*(mean+eps)), col = 32*hp + 16*h2 + chunk
        nkcols = const_pool.tile([128, 8 * NTS], F32, name="nkcols")

        # ====== phase 1: QKV projections + rotary + QK-RMSNorm ==========
        with ExitStack() as ph1:
            w_pool = ph1.enter_context(tc.tile_pool(name="wqkv", bufs=1))
            x_pool = ph1.enter_context(tc.tile_pool(name="xs", bufs=10))
            rc_pool = ph1.enter_context(tc.tile_pool(name="rotc", bufs=1))
            t_pool = ph1.enter_context(tc.tile_pool(name="rott", bufs=2))
            ps_p = ph1.enter_context(tc.tile_pool(name="psp", bufs=3, space="PSUM"))
            ps_x = ph1.enter_context(tc.tile_pool(name="psx", bufs=2, space="PSUM"))
            ps_m = ph1.enter_context(tc.tile_pool(name="psm", bufs=2, space="PSUM"))
            ps_nk = ph1.enter_context(
                tc.tile_pool(name="psnk", bufs=1, space="PSUM"))

            wq_s = [w_pool.tile([128, DH], F32, name=f"wq{k}") for k in range(NKC)]
            wk_s = [w_pool.tile([128, DH], F32, name=f"wk{k}") for k in range(NKC)]
            wv_s = [w_pool.tile([128, DH], F32, name=f"wv{k}") for k in range(NKC)]
            for k in range(NKC):
                ksl = slice(k * 128, (k + 1) * 128)
                nc.sync.dma_start(_r(wq_s[k][:]), _r(wqt[ksl, :]))
                nc.sync.dma_start(_r(wk_s[k][:]), _r(wkt[ksl, :]))
                nc.sync.dma_start(_r(wv_s[k][:]), _r(wvt[ksl, :]))
            c2 = rc_pool.tile([128, T], F32, name="c2_s")
            s2 = rc_pool.tile([128, T], F32, name="s2_s")
            nc.sync.dma_start(c2[:], c2d[:, 0:T])
            nc.sync.dma_start(s2[:], s2d[:, 0:T])
            nkp = ps_nk.tile([128, 8 * NTS], F32, name="nkp")

            for tt in range(NTT):
                tsl = slice(tt * 512, (tt + 1) * 512)
                xts = []
                for k in range(NKC):
                    xc = x_pool.tile([128, 512], F32, name="xc", tag="xc")
                    nc.gpsimd.dma_start(_r(xc[:]),
                                        _r(xt[k * 128:(k + 1) * 128, tsl]))
                    xts.append(xc)
                # Qt / Kt: psum[j_loc, t] = sum_d w[d, j] * x[d, t]
                for (wsrc, dst) in ((wq_s, qt_s), (wk_s, kt_s)):
                    for jt in range(4):
                        jsl = slice(jt * 128, (jt + 1) * 128)
                        pp = ps_p.tile([128, 512], F32, name="pp", tag="pp")
                        for k in range(NKC):
                            nc.tensor.matmul(
                                pp[:], _r(wsrc[k][:, jsl]), _r(xts[k][:]),
                                start=(k == 0), stop=(k == NKC - 1))
                        nc.vector.tensor_copy(_r(dst[jt][:, tsl]), pp[:])
                # V: psum[t_loc, j] = sum_d x[d, t] * wv[d, j]
                for ts_ in range(4):
                    ci = tt * 4 + ts_
                    pv = ps_p.tile([128, 512], F32, name="pv", tag="pp")
                    for k in range(NKC):
                        nc.tensor.matmul(
                            pv[:], _r(xts[k][:, ts_ * 128:(ts_ + 1) * 128]),
                            _r(wv_s[k][:]),
                            start=(k == 0), stop=(k == NKC - 1))
                    v3 = v_s[ci].rearrange("p (h e) -> p h e", h=NH)
                    nc.vector.tensor_copy(
                        _r(v3[:, :, 0:64]), pv.rearrange("p (h e) -> p h e", h=NH))
                    nc.vector.tensor_copy(_r(v3[:, :, 64:65]),
                                          onesc[:].unsqueeze(-1))
                # rotary + norm on the just-finished 512-block of each tile
                bsl = tsl
                for jt in range(4):
                    # Q: full norm multiply (nq varies along the scores' free
                    # dim i, so it must be applied to Q itself)
                    q = qt_s[jt]
                    xsq = ps_x.tile([128, 512], F32, name="xsq", tag="xs")
                    nc.tensor.matmul(xsq[:], _r(pswap[:]), _r(q[:, bsl]),
                                     start=True, stop=True)
                    sq = t_pool.tile([128, 512], F32, name="sq", tag="sq")
                    nc.scalar.activation(_r(sq[:]), q[:, bsl], AF.Square)
                    ms = ps_m.tile([128, 512], F32, name="ms", tag="ms")
                    nc.tensor.matmul(ms[:], _r(bdiag[:]), _r(sq[:]),
                                     start=True, stop=True)
                    s1 = t_pool.tile([128, 512], F32, name="s1", tag="s1")
                    nc.scalar.activation(s1[:], ms[:], AF.Sqrt,
                                         scale=0.125, bias=epsb[:])
                    nc.vector.reciprocal_approx_fast(out=s1[:], in_=s1[:])
                    nc.gpsimd.tensor_mul(_r(q[:, bsl]), q[:, bsl], c2[:, bsl])
                    nc.vector.tensor_mul(xsq[:], xsq[:], s2[:, bsl])
                    nc.vector.tensor_add(_r(q[:, bsl]), q[:, bsl], xsq[:])
                    nc.gpsimd.tensor_mul(_r(q[:, bsl]), q[:, bsl], s1[:])
                    # K: rotary only; nk[j] is applied later as exp()'s
                    # per-partition scale. Sumsq via tiny N=1 matmuls.
                    k_ = kt_s[jt]
                    xsk = ps_x.tile([128, 512], F32, name="xsk", tag="xs")
                    nc.tensor.matmul(xsk[:], _r(pswap[:]), _r(k_[:, bsl]),
                                     start=True, stop=True)
                    sqk = t_pool.tile([128, 512], F32, name="sqk", tag="sq")
                    nc.scalar.activation(_r(sqk[:]), k_[:, bsl], AF.Square)
                    for h2 in range(2):
                        for c4 in range(4):
                            col = (2 * jt + h2) * NTS + tt * 4 + c4
                            nc.tensor.matmul(
                                nkp[:, col:col + 1],
                                sqk[h2 * 64:h2 * 64 + 64,
                                    c4 * 128:(c4 + 1) * 128],
                                ones64[h2 * 64:h2 * 64 + 64, :],
                                start=True, stop=True)
                    nc.gpsimd.tensor_mul(_r(k_[:, bsl]), k_[:, bsl], c2[:, bsl])
                    nc.vector.tensor_mul(xsk[:], xsk[:], s2[:, bsl])
                    nc.vector.tensor_add(_r(k_[:, bsl]), k_[:, bsl], xsk[:])
            s1k = t_pool.tile([128, 8 * NTS], F32, name="s1k", tag="s1k")
            nc.scalar.activation(s1k[:], nkp[:], AF.Sqrt,
                                 scale=0.125, bias=epsb[:])
            nc.vector.reciprocal_approx_fast(out=nkcols[:], in_=s1k[:])

        if dbg:  # rotated+normed Q
            for j in range(4):
                nc.sync.dma_start(dqr[j], qt_s[j][:])

        # =================== phase 2: attention =========================
        # Ot assembled, normalized: 4 tiles [128, T] = 512 head-dims
        ot_pool = ctx.enter_context(tc.tile_pool(name="otf", bufs=1))
        wo_pool = ctx.enter_context(tc.tile_pool(name="wo", bufs=1))
        otf = [ot_pool.tile([128, T], F32, name=f"otf{j}") for j in range(4)]
        wot_s = [wo_pool.tile([128, D], F32, name=f"wot{j}") for j in range(4)]
        for k4 in range(4):
            nc.sync.dma_start(_r(wot_s[k4][:]),
                              _r(wot[k4 * 128:(k4 + 1) * 128, :]))
        with ExitStack() as ph2:
            m_pool = ph2.enter_context(tc.tile_pool(name="mask", bufs=1))
            st_pool = ph2.enter_context(tc.tile_pool(name="stg", bufs=3))
            p_pool = ph2.enter_context(tc.tile_pool(name="pexp", bufs=3))
            e_pool = ph2.enter_context(tc.tile_pool(name="epi", bufs=4))
            ps_sc = ph2.enter_context(tc.tile_pool(name="pssc", bufs=1, space="PSUM"))
            ps_ot = ph2.enter_context(tc.tile_pool(name="psot", bufs=4, space="PSUM"))

            tri = m_pool.tile([128, 128], F32, name="tri_s")
            nc.sync.dma_start(tri[:], trid[:, 0:128])
            zer = m_pool.tile([128, 384], F32, name="zer_s")
            nc.gpsimd.memset(zer[:], 0.0)

            # head pairs (2hp, 2hp+1) share a qt/kt tile at partition 0/64;
            # their scores matmuls are emitted adjacently so the PE runs them
            # concurrently in distinct row-groups (K=64 each). i is processed
            # in 1024-wide halves so psum fits deeper pipelining.
            HWID = min(1024, T)
            NBLK = HWID // 512
            for ihalf in range(T // HWID):
                for hp in range(4):
                    ilo = ihalf * HWID
                    nj = (ilo + HWID) // 128   # chunks with j < ihi
                    ot_ps = {(h2, b): ps_ot.tile([128, 512], F32,
                                                 name="otp", tag="otp")
                             for h2 in range(2) for b in range(NBLK)}
                    for jt in range(nj):
                        jsl = slice(jt * 128, (jt + 1) * 128)
                        off0 = max(0, jt * 128 - ilo)  # diag offset in half
                        b0 = off0 // 512               # first live block
                        gap = off0 - b0 * 512
                        scs, ps = {}, {}
                        for h2 in range(2):
                            scs[h2] = ps_sc.tile([128, HWID], F32,
                                                 name="sc", tag=f"sc{h2}")
                        for sub in range(b0, NBLK):
                            ssl = slice(sub * 512, (sub + 1) * 512)
                            isl = slice(ilo + sub * 512, ilo + (sub + 1) * 512)
                            for h2 in range(2):
                                ho = h2 * 64
                                nc.tensor.matmul(
                                    scs[h2][:, ssl],
                                    _r(kt_s[hp][ho:ho + 64, jsl]),
                                    _r(qt_s[hp][ho:ho + 64, isl]),
                                    start=True, stop=True)
                        for h2 in range(2):
                            h = 2 * hp + h2
                            c_ = (2 * hp + h2) * NTS + jt
                            nk = nkcols[:, c_:c_ + 1]
                            p = p_pool.tile([128, HWID], F32, name="p",
                                            tag=f"p{h2}")
                            ps[h2] = p
                            if gap:
                                nc.gpsimd.tensor_copy(
                                    _r(p[:, b0 * 512:b0 * 512 + gap]),
                                    zer[:, 0:gap])
                            nc.scalar.activation(
                                _r(p[:, off0:HWID]), scs[h2][:, off0:HWID],
                                AF.Exp, scale=nk[:])
                            if off0 < HWID and jt * 128 >= ilo:
                                # causal mask on the diagonal 128-wide block
                                nc.gpsimd.tensor_mul(
                                    _r(p[:, off0:off0 + 128]),
                                    p[:, off0:off0 + 128], tri[:, 0:128])
                            if dbg and h == 0 and ihalf == 0 and jt in (0, 5):
                                di = 0 if jt == 0 else 1
                                nc.sync.dma_start(
                                    dp[di][:, b0 * 512:HWID],
                                    p[:, b0 * 512:HWID])
                        for h2 in range(2):
                            h = 2 * hp + h2
                            for b in range(b0, NBLK):
                                ib = NBLK * ihalf + b
                                nc.tensor.matmul(
                                    ot_ps[(h2, b)][0:65, :],
                                    _r(v_s[jt][:, 65 * h:65 * h + 65]),
                                    _r(ps[h2][:, b * 512:(b + 1) * 512]),
                                    start=(jt == 0), stop=(jt == 4 * ib + 3))
                    # epilogue: divide by denominator row (psum row 64)
                    for h2 in range(2):
                        h = 2 * hp + h2
                        ho = h2 * 64
                        for b in range(NBLK):
                            ib = NBLK * ihalf + b
                            op = ot_ps[(h2, b)]
                            # recip_approx_fast mishandles base_partition != 0
                            # on HW: stage the denom row to base-0 sbuf first.
                            den = e_pool.tile([1, 512], F32, name="den",
                                              tag="den")
                            nc.vector.tensor_copy(den[:], op[64:65, :])
                            rden = e_pool.tile([1, 512], F32, name="rden",
                                               tag="rden")
                            nc.vector.reciprocal_approx_fast(out=rden[:],
                                                             in_=den[:])
                            rb = e_pool.tile([64, 512], F32, name="rb",
                                             tag="rb")
                            nc.gpsimd.partition_broadcast(rb[:], rden[:],
                                                          channels=64)
                            nc.vector.tensor_mul(
                                _r(otf[hp][ho:ho + 64,
                                           ib * 512:(ib + 1) * 512]),
                                op[0:64, :], rb[:])

        if dbg:
            for j in range(4):
                nc.sync.dma_start(dot[j], otf[j][:])

        # ================ phase 3: output projection ====================
        with ExitStack() as ph3:
            st3_pool = ph3.enter_context(tc.tile_pool(name="stg3", bufs=3))
            ps_y = ph3.enter_context(tc.tile_pool(name="psy", bufs=3, space="PSUM"))
            for tt in range(NTT):
                tsl = slice(tt * 512, (tt + 1) * 512)
                for dt_ in range(8):
                    dsl = slice(dt_ * 128, (dt_ + 1) * 128)
                    py = ps_y.tile([128, 512], F32, name="py", tag="py")
                    for k4 in range(4):
                        nc.tensor.matmul(py[:], _r(wot_s[k4][:, dsl]),
                                         _r(otf[k4][:, tsl]),
                                         start=(k4 == 0), stop=(k4 == 3))
                    st = st3_pool.tile([128, 512], F32, name="st", tag="st")
                    nc.scalar.copy(st[:], py[:])
                    nc.sync.dma_start(yt[dsl, tsl], st[:])
    return nc


# ---------------- host-side tables & shard prep -------------------------

def host_tables(T: int = 2048):
    n = HD // 4
    af = (1.0 / 1024) ** np.linspace(0, 1, n, dtype=np.float32)
    af = np.concatenate([af, np.zeros(n, np.float32)])  # [32]
    theta = np.outer(np.arange(T, dtype=np.float32), af)  # [T, 32]
    cosT = np.cos(theta).T.astype(np.float32)  # [32, T]
    sinT = np.sin(theta).T.astype(np.float32)
    c2 = np.tile(cosT, (4, 1))                             # [128, T]
    s2 = np.tile(np.concatenate([sinT, -sinT], 0), (2, 1))  # [128, T]
    km = np.arange(128)
    pswap = (km[:, None] == (km[None, :] ^ 32)).astype(np.float32)
    bdiag = ((km[:, None] // 64) == (km[None, :] // 64)).astype(np.float32)
    tri = np.zeros((128, 2048), np.float32)
    r_ = np.arange(128)[:, None]
    c_ = np.arange(512)[None, :]
    for v in range(4):
        tri[:, 512 * v:512 * (v + 1)] = (c_ >= 128 * v + r_).astype(np.float32)
    return {"c2": np.ascontiguousarray(c2), "s2": np.ascontiguousarray(s2),
            "pswap": pswap, "bdiag": bdiag, "trimask": tri}


def core_inputs(x, wq, wk, wv, wo, core: int, T: int = 2048):
    b, g = core % 4, core // 4
    sl = slice(g * DH, (g + 1) * DH)
    m = {
        "xt": np.ascontiguousarray(np.asarray(x[b]).T.astype(np.float32)),
        "wqt": np.ascontiguousarray(np.asarray(wq)[sl, :].T.astype(np.float32)),
        "wkt": np.ascontiguousarray(np.asarray(wk)[sl, :].T.astype(np.float32)),
        "wvt": np.ascontiguousarray(np.asarray(wv)[sl, :].T.astype(np.float32)),
        "wot": np.ascontiguousarray(np.asarray(wo)[:, sl].T.astype(np.float32)),
    }
    m.update(host_tables(T))
    return m


_CACHE = {}


def _get_nc(T: int = 2048):
    key = ("nc", T)
    if key not in _CACHE:
        nc = bacc.Bacc("TRN2", target_bir_lowering=False, debug=False)
        build_kernel(nc, T)
        nc.compile()
        _CACHE[key] = nc
    return _CACHE[key]


def kernel(x, wq, wk, wv, wo, mask=None):
    from concourse import bass_utils
    nc = _get_nc(2048)
    in_maps = [core_inputs(x, wq, wk, wv, wo, c) for c in range(8)]
    res = bass_utils.run_bass_kernel_spmd(nc, in_maps, list(range(8)))
    outs = [np.asarray(res.results[c]["yt"]) for c in range(8)]
    out = np.empty((4, 2048, 1024), np.float32)
    for b in range(4):
        out[b] = (outs[b] + outs[b + 4]).T
    return out



# revision 19
# speedup vs baseline: 1.7131x; 1.7131x over previous
"""Trainium2 Bass kernel: MultiHeadAttention with QK-RMSNorm + partial rotary,
causal softmax. B=4, T=2048, D=1024, H=16, HD=64, fp32.

Sharding: 8 cores = 4 batches x 2 head-groups (8 heads each). Each core:
  - QKV projections for its batch, restricted to its 512 head-dims
  - causal attention for its 8 heads
  - partial output projection (its 512 contraction dims, all 1024 outputs)
Host sums the two head-group partials per batch (the all-reduce) and
transposes back.

v2 design (vs baseline): phase-interleaved, engine-specialized.
  - QKV block tt and attention i-block ib=tt alternate so the ACT-heavy
    softmax overlaps the PE-heavy projections; PE never idles >1us, which
    keeps the HAM clock gate at K=8/8 (fp32r at low clock is 4x slower).
  - gpsimd is NOT used for compute at all: every gpsimd ucode-kernel switch
    costs a ~6us IRAM reload that stalled the whole baseline pipeline.
    Elementwise work goes to DVE, psum->sbuf copies to ACT, the softmax
    denominator broadcast to a K=1 PE matmul.
  - ACT functions are confined to {Square, Sqrt} (one table) + Exp/Copy
    (one table) to avoid 1.28us ACT table reloads.
  - V, softmax weights, attention output and wo run in bf16 (PE rate is
    identical to fp32r; it halves SBUF/DVE cost and dodges the fp32r
    small-N 4x penalty on narrow diagonal AV matmuls).
  - causal diagonal blocks compute/exp only the valid column strip.
"""

import numpy as np
from contextlib import ExitStack

import concourse.bass as bass
import concourse.tile as tile
import concourse.mybir as mybir
from concourse import bacc

F32 = mybir.dt.float32
BF16 = mybir.dt.bfloat16
MM_DT = mybir.dt.float32r
AF = mybir.ActivationFunctionType

D = 1024   # model dim
DH = 512   # head-group width per core (8 heads x 64)
NH = 8     # heads per core
HD = 64    # head dim
NKC = D // 128   # k-chunks over model dim
EPS = 1e-6


def _r(ap):
    return ap.bitcast(MM_DT)


def build_kernel(nc: bass.Bass, T: int = 2048):
    NTT = T // 512     # 512-wide t/i blocks
    NTS = T // 128     # 128-wide t/j chunks

    xt = nc.dram_tensor("xt", [D, T], F32, kind="ExternalInput").ap()
    wqt = nc.dram_tensor("wqt", [D, DH], F32, kind="ExternalInput").ap()
    wkt = nc.dram_tensor("wkt", [D, DH], F32, kind="ExternalInput").ap()
    wvt = nc.dram_tensor("wvt", [D, DH], F32, kind="ExternalInput").ap()
    wot = nc.dram_tensor("wot", [DH, D], BF16, kind="ExternalInput").ap()
    c2d = nc.dram_tensor("c2", [128, T], F32, kind="ExternalInput").ap()
    s2d = nc.dram_tensor("s2", [128, T], F32, kind="ExternalInput").ap()
    pswapd = nc.dram_tensor("pswap", [128, 128], F32, kind="ExternalInput").ap()
    bdiagd = nc.dram_tensor("bdiag", [128, 128], F32, kind="ExternalInput").ap()
    trid = nc.dram_tensor("trimask", [128, 128], F32, kind="ExternalInput").ap()
    yt = nc.dram_tensor("yt", [D, T], F32, kind="ExternalOutput").ap()

    with tile.TileContext(nc) as tc, ExitStack() as ctx:
        # ---- persistent SBUF pools --------------------------------------
        qk_pool = ctx.enter_context(tc.tile_pool(name="qk", bufs=1))
        v_pool = ctx.enter_context(tc.tile_pool(name="v", bufs=1))
        ot_pool = ctx.enter_context(tc.tile_pool(name="otf", bufs=1))
        const_pool = ctx.enter_context(tc.tile_pool(name="const", bufs=1))
        w_pool = ctx.enter_context(tc.tile_pool(name="wqkv", bufs=1))
        wo_pool = ctx.enter_context(tc.tile_pool(name="wo", bufs=1))
        x_pool = ctx.enter_context(tc.tile_pool(name="xs", bufs=8))
        t_pool = ctx.enter_context(tc.tile_pool(name="rott", bufs=2))
        p_pool = ctx.enter_context(tc.tile_pool(name="pexp", bufs=4))
        e_pool = ctx.enter_context(tc.tile_pool(name="epi", bufs=1))
        st_pool = ctx.enter_context(tc.tile_pool(name="stg3", bufs=1))
        # single PSUM pool; tags share the 8 banks across phases:
        #   A: proj accum + attention scores      (3 banks)
        #   B: rotary pswap out + attn AV accum   (2 banks)
        #   C: rotary bdiag out + denom bcast + out-proj accum (2 banks)
        #   nk: per-tt key sumsq accum            (1 bank)
        ps = ctx.enter_context(tc.tile_pool(name="ps", bufs=1, space="PSUM"))

        qt_s = [qk_pool.tile([128, T], F32, name=f"qt{j}") for j in range(4)]
        kt_s = [qk_pool.tile([128, T], F32, name=f"kt{j}") for j in range(4)]
        v_s = [v_pool.tile([128, NH * 65], BF16, name=f"vt{j}")
               for j in range(NTS)]
        otf = [ot_pool.tile([128, T], BF16, name=f"otf{j}") for j in range(4)]

        pswap = const_pool.tile([128, 128], F32, name="pswap_s")
        bdiag = const_pool.tile([128, 128], F32, name="bdiag_s")
        trif = const_pool.tile([128, 128], F32, name="trif_s")
        nc.sync.dma_start(_r(pswap[:]), _r(pswapd[:]))
        nc.sync.dma_start(_r(bdiag[:]), _r(bdiagd[:]))
        nc.sync.dma_start(trif[:], trid[:])
        epsb = const_pool.tile([128, 1], F32, name="epsb")
        nc.vector.memset(epsb[:], 8.0 * EPS)
        onescb = const_pool.tile([128, NH], BF16, name="onescb")
        nc.vector.memset(onescb[:], 1.0)
        ones64 = const_pool.tile([128, 1], F32, name="ones64")
        nc.vector.memset(ones64[:], 1.0)
        # nk columns: rsqrt(8*(mean+eps)) per key pos; col = 8*jchunk + head
        nkcols = const_pool.tile([128, 8 * NTS], F32, name="nkcols")
        c2 = const_pool.tile([128, T], F32, name="c2_s")
        s2 = const_pool.tile([128, T], F32, name="s2_s")
        nc.sync.dma_start(c2[:], c2d[:])
        nc.sync.dma_start(s2[:], s2d[:])

        # QKV weights (f32; bitcast f32r at use)
        wq_s = [w_pool.tile([128, DH], F32, name=f"wq{k}") for k in range(NKC)]
        wk_s = [w_pool.tile([128, DH], F32, name=f"wk{k}") for k in range(NKC)]
        wv_s = [w_pool.tile([128, DH], F32, name=f"wv{k}") for k in range(NKC)]
        for k in range(NKC):
            ksl = slice(k * 128, (k + 1) * 128)
            nc.sync.dma_start(_r(wq_s[k][:]), _r(wqt[ksl, :]))
            nc.sync.dma_start(_r(wk_s[k][:]), _r(wkt[ksl, :]))
            nc.sync.dma_start(_r(wv_s[k][:]), _r(wvt[ksl, :]))
        # wo arrives pre-cast to bf16 from the host
        wob = [wo_pool.tile([128, D], BF16, name=f"wob{k}") for k in range(4)]
        for k4 in range(4):
            nc.sync.dma_start(wob[k4][:], wot[k4 * 128:(k4 + 1) * 128, :])

        for tt in range(NTT):
            tsl = slice(tt * 512, (tt + 1) * 512)
            # ---- x chunks for this block -------------------------------
            xts = []
            for k in range(NKC):
                xc = x_pool.tile([128, 512], F32, name="xc", tag="xc")
                nc.sync.dma_start(_r(xc[:]),
                                  _r(xt[k * 128:(k + 1) * 128, tsl]))
                xts.append(xc)

            # ---- Q projection + rotary + RMS-norm, per head pair -------
            for hp in range(4):
                jsl = slice(hp * 128, (hp + 1) * 128)
                pp = ps.tile([128, 512], F32, name="pp", tag="A", bufs=3)
                for k in range(NKC):
                    nc.tensor.matmul(pp[:], _r(wq_s[k][:, jsl]), _r(xts[k][:]),
                                     start=(k == 0), stop=(k == NKC - 1))
                q = qt_s[hp]
                nc.scalar.copy(_r(q[:, tsl]), pp[:])
                sq = t_pool.tile([128, 512], F32, name="sq", tag="sq")
                nc.scalar.activation(_r(sq[:]), pp[:], AF.Square)
                xsq = ps.tile([128, 512], F32, name="xsq", tag="B", bufs=2)
                nc.tensor.matmul(xsq[:], _r(pswap[:]), _r(q[:, tsl]),
                                 start=True, stop=True)
                ms = ps.tile([128, 512], F32, name="ms", tag="C", bufs=2)
                nc.tensor.matmul(ms[:], _r(bdiag[:]), _r(sq[:]),
                                 start=True, stop=True)
                s1 = t_pool.tile([128, 512], F32, name="s1", tag="s1")
                nc.scalar.activation(s1[:], ms[:], AF.Sqrt,
                                     scale=0.125, bias=epsb[:])
                nc.vector.reciprocal_approx_fast(out=s1[:], in_=s1[:])
                nc.vector.tensor_mul(_r(q[:, tsl]), q[:, tsl], c2[:, tsl])
                nc.vector.tensor_mul(xsq[:], xsq[:], s2[:, tsl])
                nc.vector.tensor_add(_r(q[:, tsl]), q[:, tsl], xsq[:])
                nc.vector.tensor_mul(_r(q[:, tsl]), q[:, tsl], s1[:])

            # ---- K projection + rotary (norm folded into exp scale) ----
            nkp = ps.tile([128, 32], F32, name="nkp", tag="nk", bufs=1)
            for hp in range(4):
                jsl = slice(hp * 128, (hp + 1) * 128)
                pp = ps.tile([128, 512], F32, name="ppk", tag="A", bufs=3)
                for k in range(NKC):
                    nc.tensor.matmul(pp[:], _r(wk_s[k][:, jsl]), _r(xts[k][:]),
                                     start=(k == 0), stop=(k == NKC - 1))
                k_ = kt_s[hp]
                nc.scalar.copy(_r(k_[:, tsl]), pp[:])
                sqk = t_pool.tile([128, 512], F32, name="sqk", tag="sq")
                nc.scalar.activation(_r(sqk[:]), pp[:], AF.Square)
                xsk = ps.tile([128, 512], F32, name="xsk", tag="B", bufs=2)
                nc.tensor.matmul(xsk[:], _r(pswap[:]), _r(k_[:, tsl]),
                                 start=True, stop=True)
                # key sumsq -> nkp columns (col = 8*(j local chunk) + head)
                for h2 in range(2):
                    for c4 in range(4):
                        col = c4 * 8 + 2 * hp + h2
                        nc.tensor.matmul(
                            nkp[:, col:col + 1],
                            sqk[h2 * 64:h2 * 64 + 64,
                                c4 * 128:(c4 + 1) * 128],
                            ones64[h2 * 64:h2 * 64 + 64, :],
                            start=True, stop=True)
                nc.vector.tensor_mul(_r(k_[:, tsl]), k_[:, tsl], c2[:, tsl])
                nc.vector.tensor_mul(xsk[:], xsk[:], s2[:, tsl])
                nc.vector.tensor_add(_r(k_[:, tsl]), k_[:, tsl], xsk[:])

            # ---- V projection (transposed) -----------------------------
            for ts_ in range(4):
                ci = tt * 4 + ts_
                pv = ps.tile([128, 512], F32, name="pv", tag="A", bufs=3)
                for k in range(NKC):
                    nc.tensor.matmul(
                        pv[:], _r(xts[k][:, ts_ * 128:(ts_ + 1) * 128]),
                        _r(wv_s[k][:]),
                        start=(k == 0), stop=(k == NKC - 1))
                v3 = v_s[ci].rearrange("p (h e) -> p h e", h=NH)
                nc.scalar.copy(v3[:, :, 0:64],
                               pv.rearrange("p (h e) -> p h e", h=NH))
                nc.vector.tensor_copy(v3[:, :, 64:65], onescb[:].unsqueeze(-1))

            # nk finalize for this tt's 4 chunks (32 cols)
            s1k = t_pool.tile([128, 32], F32, name="s1k", tag="s1k")
            nc.scalar.activation(s1k[:], nkp[:], AF.Sqrt,
                                 scale=0.125, bias=epsb[:])
            nc.vector.reciprocal_approx_fast(
                out=nkcols[:, 32 * tt:32 * tt + 32], in_=s1k[:])

            # ================= attention for i-block ib = tt ============
            ib = tt
            isl = tsl
            nj = 4 * ib + 4
            for hp in range(4):
                ot_ps = [ps.tile([128, 512], F32, name="otp", tag="B", bufs=2)
                         for _ in range(2)]
                for jt in range(nj):
                    jsl = slice(jt * 128, (jt + 1) * 128)
                    c_ = jt - 4 * ib          # >=0 on diagonal chunks
                    off = 128 * c_ if c_ >= 0 else 0
                    osc = off if off <= 256 else 256   # keep f32r N>=256
                    scs, pexp = [], []
                    for h2 in range(2):
                        sc = ps.tile([128, 512], F32, name="sc", tag="A",
                                     bufs=3)
                        scs.append(sc)
                        ho = h2 * 64
                        nc.tensor.matmul(
                            sc[:, osc:512],
                            _r(kt_s[hp][ho:ho + 64, jsl]),
                            _r(qt_s[hp][ho:ho + 64,
                                        ib * 512 + osc:ib * 512 + 512]),
                            start=True, stop=True)
                    for h2 in range(2):
                        h = 2 * hp + h2
                        nk = nkcols[:, 8 * jt + h:8 * jt + h + 1]
                        p = p_pool.tile([128, 512], BF16, name="p", tag="p")
                        pexp.append(p)
                        if c_ >= 0:
                            # additive causal mask (-300 below diag) pre-exp
                            nc.vector.tensor_add(scs[h2][:, off:off + 128],
                                                 scs[h2][:, off:off + 128],
                                                 trif[:])
                        nc.scalar.activation(p[:, off:512], scs[h2][:, off:512],
                                             AF.Exp, scale=nk)
                    for h2 in range(2):
                        h = 2 * hp + h2
                        nc.tensor.matmul(
                            ot_ps[h2][0:65, off:512],
                            v_s[jt][:, 65 * h:65 * h + 65],
                            pexp[h2][:, off:512],
                            start=(jt == 0), stop=(jt == nj - 1))
                # epilogue: divide by the denominator row (psum row 64).
                # partition_broadcast is the ONLY gpsimd ucode kernel in the
                # program, so its IRAM load is paid once.
                for h2 in range(2):
                    op = ot_ps[h2]
                    den = e_pool.tile([1, 512], F32, name="den", tag="den")
                    nc.vector.tensor_copy(den[:], op[64:65, :])
                    rden = e_pool.tile([1, 512], F32, name="rden", tag="rden")
                    nc.vector.reciprocal_approx_fast(out=rden[:], in_=den[:])
                    rb = e_pool.tile([64, 512], F32, name="rb", tag="rb")
                    nc.gpsimd.partition_broadcast(rb[:], rden[:], channels=64)
                    ho = h2 * 64
                    nc.vector.tensor_mul(otf[hp][ho:ho + 64, isl],
                                         op[0:64, :], rb[:])

        # ================ output projection =============================
        for tt in range(NTT):
            tsl = slice(tt * 512, (tt + 1) * 512)
            for dt_ in range(8):
                dsl = slice(dt_ * 128, (dt_ + 1) * 128)
                py = ps.tile([128, 512], F32, name="py", tag="C", bufs=2)
                for k4 in range(4):
                    nc.tensor.matmul(py[:], wob[k4][:, dsl],
                                     otf[k4][:, tsl],
                                     start=(k4 == 0), stop=(k4 == 3))
                st = st_pool.tile([128, 512], F32, name="st", tag="st")
                nc.scalar.copy(st[:], py[:])
                nc.sync.dma_start(yt[dsl, tsl], st[:])
    return nc


# ---------------- host-side tables & shard prep -------------------------

def host_tables(T: int = 2048):
    n = HD // 4
    af = (1.0 / 1024) ** np.linspace(0, 1, n, dtype=np.float32)
    af = np.concatenate([af, np.zeros(n, np.float32)])  # [32]
    theta = np.outer(np.arange(T, dtype=np.float32), af)  # [T, 32]
    cosT = np.cos(theta).T.astype(np.float32)  # [32, T]
    sinT = np.sin(theta).T.astype(np.float32)
    c2 = np.tile(cosT, (4, 1))                             # [128, T]
    s2 = np.tile(np.concatenate([sinT, -sinT], 0), (2, 1))  # [128, T]
    km = np.arange(128)
    pswap = (km[:, None] == (km[None, :] ^ 32)).astype(np.float32)
    bdiag = ((km[:, None] // 64) == (km[None, :] // 64)).astype(np.float32)
    r_ = np.arange(128)[:, None]
    c_ = np.arange(128)[None, :]
    tri = np.where(c_ >= r_, 0.0, -300.0).astype(np.float32)
    return {"c2": np.ascontiguousarray(c2), "s2": np.ascontiguousarray(s2),
            "pswap": pswap, "bdiag": bdiag, "trimask": tri}


def _bf16():
    import ml_dtypes
    return ml_dtypes.bfloat16


def core_inputs(x, wq, wk, wv, wo, core: int, T: int = 2048):
    b, g = core % 4, core // 4
    sl = slice(g * DH, (g + 1) * DH)
    m = {
        "xt": np.ascontiguousarray(np.asarray(x[b]).T.astype(np.float32)),
        "wqt": np.ascontiguousarray(np.asarray(wq)[sl, :].T.astype(np.float32)),
        "wkt": np.ascontiguousarray(np.asarray(wk)[sl, :].T.astype(np.float32)),
        "wvt": np.ascontiguousarray(np.asarray(wv)[sl, :].T.astype(np.float32)),
        "wot": np.ascontiguousarray(
            np.asarray(wo)[:, sl].T.astype(_bf16())),
    }
    m.update(host_tables(T))
    return m


_CACHE = {}


def _get_nc(T: int = 2048):
    key = ("nc", T)
    if key not in _CACHE:
        nc = bacc.Bacc("TRN2", target_bir_lowering=False, debug=False)
        build_kernel(nc, T)
        nc.compile()
        _CACHE[key] = nc
    return _CACHE[key]


def kernel(x, wq, wk, wv, wo, mask=None):
    from concourse import bass_utils
    nc = _get_nc(2048)
    in_maps = [core_inputs(x, wq, wk, wv, wo, c) for c in range(8)]
    res = bass_utils.run_bass_kernel_spmd(nc, in_maps, list(range(8)))
    outs = [np.asarray(res.results[c]["yt"]) for c in range(8)]
    out = np.empty((4, 2048, 1024), np.float32)
    for b in range(4):
        out[b] = (outs[b] + outs[b + 4]).T
    return out


# revision 27
# speedup vs baseline: 1.9258x; 1.1241x over previous
"""Trainium2 Bass kernel: MultiHeadAttention with QK-RMSNorm + partial rotary,
causal softmax. B=4, T=2048, D=1024, H=16, HD=64, fp32.

Sharding: 8 cores = 4 batches x 2 head-groups (8 heads each). Each core:
  - QKV projections for its batch, restricted to its 512 head-dims
  - causal attention for its 8 heads
  - partial output projection (its 512 contraction dims, all 1024 outputs)
Host sums the two head-group partials per batch (the all-reduce) and
transposes back.

v3 design: phase-interleaved, engine-specialized, single-exp scores.
  - Per 512-token block tt: QKV projections -> rotary+RMS-norm -> attention
    for i-block tt -> output projection of i-block tt. The ACT-heavy softmax
    and the DMA-heavy output store overlap the PE-heavy projections of the
    next block, keeping the PE dense so the HAM clock gate stays at K=8/8
    (fp32r at low clock is 4x slower).
  - K is normalized exactly like Q (bdiag matmul + sqrt + recip + mul), so
    exp needs no per-head scale and one [128,1024] exp op covers both heads
    of a pair (halves ACT instruction count - ACT is the attention pacer).
  - gpsimd runs ONLY partition_broadcast (softmax denominator): any second
    gpsimd ucode kernel would trigger ~6us IRAM reloads per switch.
  - ACT tables: {Square,Sqrt,Copy} + {Exp,Copy} - two loads per block max.
  - V, softmax weights, attention output and wo in bf16 (same PE rate as
    fp32r, halves SBUF/DVE cost, dodges fp32r small-N penalty on diagonal).
  - causal diagonal blocks compute/exp only the valid column strip; mask is
    an additive -300 on scores pre-exp.
"""

import numpy as np
from contextlib import ExitStack

import concourse.bass as bass
import concourse.tile as tile
import concourse.mybir as mybir
from concourse import bacc

F32 = mybir.dt.float32
BF16 = mybir.dt.bfloat16
MM_DT = mybir.dt.float32r
AF = mybir.ActivationFunctionType

D = 1024   # model dim
DH = 512   # head-group width per core (8 heads x 64)
NH = 8     # heads per core
HD = 64    # head dim
NKC = D // 128   # k-chunks over model dim
EPS = 1e-6


def _r(ap):
    return ap.bitcast(MM_DT)


def build_kernel(nc: bass.Bass, T: int = 2048):
    NTT = T // 512     # 512-wide t/i blocks
    NTS = T // 128     # 128-wide t/j chunks

    xt = nc.dram_tensor("xt", [D, T], F32, kind="ExternalInput").ap()
    wqt = nc.dram_tensor("wqt", [D, DH], F32, kind="ExternalInput").ap()
    wkt = nc.dram_tensor("wkt", [D, DH], F32, kind="ExternalInput").ap()
    wvt = nc.dram_tensor("wvt", [D, DH], F32, kind="ExternalInput").ap()
    wot = nc.dram_tensor("wot", [DH, D], BF16, kind="ExternalInput").ap()
    c2d = nc.dram_tensor("c2", [128, T], F32, kind="ExternalInput").ap()
    s2d = nc.dram_tensor("s2", [128, T], F32, kind="ExternalInput").ap()
    pswapd = nc.dram_tensor("pswap", [128, 128], F32, kind="ExternalInput").ap()
    bdiagd = nc.dram_tensor("bdiag", [128, 128], F32, kind="ExternalInput").ap()
    trid = nc.dram_tensor("trimask", [128, 256], F32, kind="ExternalInput").ap()
    yt = nc.dram_tensor("yt", [D, T], F32, kind="ExternalOutput").ap()

    with tile.TileContext(nc) as tc, ExitStack() as ctx:
        # ---- persistent SBUF pools --------------------------------------
        qk_pool = ctx.enter_context(tc.tile_pool(name="qk", bufs=1))
        v_pool = ctx.enter_context(tc.tile_pool(name="v", bufs=1))
        ot_pool = ctx.enter_context(tc.tile_pool(name="otf", bufs=1))
        const_pool = ctx.enter_context(tc.tile_pool(name="const", bufs=1))
        w_pool = ctx.enter_context(tc.tile_pool(name="wqkv", bufs=1))
        wo_pool = ctx.enter_context(tc.tile_pool(name="wo", bufs=1))
        x_pool = ctx.enter_context(tc.tile_pool(name="xs", bufs=8))
        cs_pool = ctx.enter_context(tc.tile_pool(name="cs", bufs=1))
        t_pool = ctx.enter_context(tc.tile_pool(name="rott", bufs=2))
        p_pool = ctx.enter_context(tc.tile_pool(name="pexp", bufs=3))
        e_pool = ctx.enter_context(tc.tile_pool(name="epi", bufs=1))
        st_pool = ctx.enter_context(tc.tile_pool(name="stg3", bufs=2))
        # single PSUM pool; tags share the 8 banks across phases:
        #   A: proj accum [128,512] + scores pair [128,1024]   (2x2 banks)
        #   B: rotary pswap out + attn AV accum                (2 banks)
        #   C: rotary bdiag out + out-proj accum               (2 banks)
        ps = ctx.enter_context(tc.tile_pool(name="ps", bufs=1, space="PSUM"))

        qt_s = [qk_pool.tile([128, T], F32, name=f"qt{j}") for j in range(4)]
        kt_s = [qk_pool.tile([128, T], F32, name=f"kt{j}") for j in range(4)]
        v_s = [v_pool.tile([128, NH * 65], BF16, name=f"vt{j}")
               for j in range(NTS)]
        otf = [ot_pool.tile([128, T], BF16, name=f"otf{j}") for j in range(4)]

        # weights first: they gate the first projection matmuls.
        wq_s = [w_pool.tile([128, DH], F32, name=f"wq{k}") for k in range(NKC)]
        wk_s = [w_pool.tile([128, DH], F32, name=f"wk{k}") for k in range(NKC)]
        wv_s = [w_pool.tile([128, DH], F32, name=f"wv{k}") for k in range(NKC)]
        for k in range(NKC):
            ksl = slice(k * 128, (k + 1) * 128)
            nc.sync.dma_start(_r(wq_s[k][:]), _r(wqt[ksl, :]))
        pswap = const_pool.tile([128, 128], F32, name="pswap_s")
        bdiag = const_pool.tile([128, 128], F32, name="bdiag_s")
        trif = const_pool.tile([128, 256], F32, name="trif_s")
        nc.scalar.dma_start(_r(pswap[:]), _r(pswapd[:]))
        nc.scalar.dma_start(_r(bdiag[:]), _r(bdiagd[:]))
        for k in range(NKC):
            ksl = slice(k * 128, (k + 1) * 128)
            nc.scalar.dma_start(_r(wk_s[k][:]), _r(wkt[ksl, :]))
            nc.scalar.dma_start(_r(wv_s[k][:]), _r(wvt[ksl, :]))
        nc.scalar.dma_start(trif[:], trid[:])
        wob = [wo_pool.tile([128, D], BF16, name=f"wob{k}") for k in range(4)]
        for k4 in range(4):
            nc.scalar.dma_start(wob[k4][:], wot[k4 * 128:(k4 + 1) * 128, :])
        epsb = const_pool.tile([128, 1], F32, name="epsb")
        nc.vector.memset(epsb[:], 8.0 * EPS)
        onescb = const_pool.tile([128, NH], BF16, name="onescb")
        nc.vector.memset(onescb[:], 1.0)

        for tt in range(NTT):
            tsl = slice(tt * 512, (tt + 1) * 512)
            # x chunks + rotary tables for this block
            xts = []
            for k in range(NKC):
                xc = x_pool.tile([128, 512], F32, name="xc", tag="xc")
                nc.sync.dma_start(_r(xc[:]),
                                  _r(xt[k * 128:(k + 1) * 128, tsl]))
                xts.append(xc)
            c2t = cs_pool.tile([128, 512], F32, name="c2t", tag="c2t")
            s2t = cs_pool.tile([128, 512], F32, name="s2t", tag="s2t")
            nc.sync.dma_start(c2t[:], c2d[:, tsl])
            nc.sync.dma_start(s2t[:], s2d[:, tsl])

            # ---- projections: PE stays dense, ACT trails with copies ---
            for (wsrc, nm) in ((wq_s, "q"), (wk_s, "k")):
                for hp in range(4):
                    jsl = slice(hp * 128, (hp + 1) * 128)
                    pp = ps.tile([128, 1024], F32, name="pp", tag="A", bufs=2)
                    for k in range(NKC):
                        nc.tensor.matmul(pp[:, 0:512], _r(wsrc[k][:, jsl]),
                                         _r(xts[k][:]),
                                         start=(k == 0), stop=(k == NKC - 1))
                    dst = (qt_s if nm == "q" else kt_s)[hp]
                    nc.scalar.copy(_r(dst[:, tsl]), pp[:, 0:512])
            for ts_ in range(4):
                ci = tt * 4 + ts_
                pv = ps.tile([128, 1024], F32, name="pv", tag="A", bufs=2)
                for k in range(NKC):
                    nc.tensor.matmul(
                        pv[:, 0:512], _r(xts[k][:, ts_ * 128:(ts_ + 1) * 128]),
                        _r(wv_s[k][:]),
                        start=(k == 0), stop=(k == NKC - 1))
                v3 = v_s[ci].rearrange("p (h e) -> p h e", h=NH)
                nc.scalar.copy(v3[:, :, 0:64],
                               pv[:, 0:512].rearrange("p (h e) -> p h e", h=NH))
                nc.vector.tensor_copy(v3[:, :, 64:65], onescb[:].unsqueeze(-1))

            # ---- rotary + RMS-norm for q and k (identical paths) -------
            # squares first (ACT, from SBUF) so the bdiag matmuls that
            # consume them never stall the PE stream below.
            for hp in range(4):
                for nm in ("q", "k"):
                    dst = (qt_s if nm == "q" else kt_s)[hp]
                    sq = t_pool.tile([128, 512], F32, name="sq", tag=f"sq{nm}")
                    nc.scalar.activation(_r(sq[:]), dst[:, tsl], AF.Square)
                    xs_ = ps.tile([128, 512], F32, name="xs", tag="B", bufs=2)
                    nc.tensor.matmul(xs_[:], _r(pswap[:]), _r(dst[:, tsl]),
                                     start=True, stop=True)
                    ms = ps.tile([128, 512], F32, name="ms", tag="C", bufs=2)
                    nc.tensor.matmul(ms[:], _r(bdiag[:]), _r(sq[:]),
                                     start=True, stop=True)
                    s1 = t_pool.tile([128, 512], F32, name="s1", tag=f"s1{nm}")
                    nc.scalar.activation(s1[:], ms[:], AF.Sqrt,
                                         scale=0.125, bias=epsb[:])
                    nc.vector.reciprocal_approx_fast(out=s1[:], in_=s1[:])
                    nc.vector.tensor_mul(_r(dst[:, tsl]), dst[:, tsl],
                                         c2t[:])
                    nc.vector.tensor_mul(xs_[:], xs_[:], s2t[:])
                    nc.vector.tensor_add(_r(dst[:, tsl]), dst[:, tsl], xs_[:])
                    nc.vector.tensor_mul(_r(dst[:, tsl]), dst[:, tsl], s1[:])

            # ================= attention for i-block ib = tt ============
            ib = tt
            isl = tsl
            nj = 4 * ib + 4
            for hp in range(4):
                ot_ps = [ps.tile([128, 512], F32, name="otp", tag="B", bufs=2)
                         for _ in range(2)]
                for jt in range(nj):
                    jsl = slice(jt * 128, (jt + 1) * 128)
                    c_ = jt - 4 * ib          # >=0 on diagonal chunks
                    off = 128 * c_ if c_ >= 0 else 0
                    osc = off if off <= 256 else 256   # keep f32r N>=256
                    sc = ps.tile([128, 1024], F32, name="sc", tag="A", bufs=2)
                    for h2 in range(2):
                        ho = h2 * 64
                        nc.tensor.matmul(
                            sc[:, 512 * h2 + osc:512 * h2 + 512],
                            _r(kt_s[hp][ho:ho + 64, jsl]),
                            _r(qt_s[hp][ho:ho + 64,
                                        ib * 512 + osc:ib * 512 + 512]),
                            start=True, stop=True)
                    sc3 = sc.rearrange("p (h e) -> p h e", h=2)
                    if c_ >= 0:
                        # additive causal mask (-300 below diag) pre-exp
                        nc.vector.tensor_add(
                            sc3[:, :, off:off + 128], sc3[:, :, off:off + 128],
                            trif[:].rearrange("p (h e) -> p h e", h=2))
                    p = p_pool.tile([128, 1024], BF16, name="p", tag="p")
                    p3 = p.rearrange("p (h e) -> p h e", h=2)
                    nc.scalar.activation(p3[:, :, off:512], sc3[:, :, off:512],
                                         AF.Exp)
                    for h2 in range(2):
                        h = 2 * hp + h2
                        nc.tensor.matmul(
                            ot_ps[h2][0:65, off:512],
                            v_s[jt][:, 65 * h:65 * h + 65],
                            p[:, 512 * h2 + off:512 * h2 + 512],
                            start=(jt == 0), stop=(jt == nj - 1))
                # epilogue: divide by the denominator row (psum row 64).
                # partition_broadcast is the ONLY gpsimd ucode kernel here,
                # so its IRAM load is paid once.
                for h2 in range(2):
                    op = ot_ps[h2]
                    den = e_pool.tile([1, 512], F32, name="den", tag="den")
                    if tt < 2:
                        nc.scalar.copy(den[:], op[64:65, :])
                    else:
                        nc.vector.tensor_copy(den[:], op[64:65, :])
                    rden = e_pool.tile([1, 512], F32, name="rden", tag="rden")
                    nc.vector.reciprocal_approx_fast(out=rden[:], in_=den[:])
                    rb = e_pool.tile([64, 512], F32, name="rb", tag="rb")
                    nc.gpsimd.partition_broadcast(rb[:], rden[:], channels=64)
                    ho = h2 * 64
                    nc.vector.tensor_mul(otf[hp][ho:ho + 64, isl],
                                         op[0:64, :], rb[:])

            # ---- output projection for this i-block --------------------
            for dt_ in range(8):
                dsl = slice(dt_ * 128, (dt_ + 1) * 128)
                py = ps.tile([128, 512], F32, name="py", tag="C", bufs=2)
                for k4 in range(4):
                    nc.tensor.matmul(py[:], wob[k4][:, dsl],
                                     otf[k4][:, tsl],
                                     start=(k4 == 0), stop=(k4 == 3))
                st = st_pool.tile([128, 512], F32, name="st", tag="st")
                if tt < 2:
                    nc.scalar.copy(st[:], py[:])
                else:
                    nc.vector.tensor_copy(st[:], py[:])
                nc.sync.dma_start(yt[dsl, tsl], st[:])
    return nc


# ---------------- host-side tables & shard prep -------------------------

def host_tables(T: int = 2048):
    n = HD // 4
    af = (1.0 / 1024) ** np.linspace(0, 1, n, dtype=np.float32)
    af = np.concatenate([af, np.zeros(n, np.float32)])  # [32]
    theta = np.outer(np.arange(T, dtype=np.float32), af)  # [T, 32]
    cosT = np.cos(theta).T.astype(np.float32)  # [32, T]
    sinT = np.sin(theta).T.astype(np.float32)
    c2 = np.tile(cosT, (4, 1))                             # [128, T]
    s2 = np.tile(np.concatenate([sinT, -sinT], 0), (2, 1))  # [128, T]
    km = np.arange(128)
    pswap = (km[:, None] == (km[None, :] ^ 32)).astype(np.float32)
    bdiag = ((km[:, None] // 64) == (km[None, :] // 64)).astype(np.float32)
    r_ = np.arange(128)[:, None]
    c_ = np.arange(128)[None, :]
    tri1 = np.where(c_ >= r_, 0.0, -300.0).astype(np.float32)
    tri = np.concatenate([tri1, tri1], axis=1)             # [128, 256]
    return {"c2": np.ascontiguousarray(c2), "s2": np.ascontiguousarray(s2),
            "pswap": pswap, "bdiag": bdiag,
            "trimask": np.ascontiguousarray(tri)}


def _bf16():
    import ml_dtypes
    return ml_dtypes.bfloat16


def core_inputs(x, wq, wk, wv, wo, core: int, T: int = 2048):
    b, g = core % 4, core // 4
    sl = slice(g * DH, (g + 1) * DH)
    m = {
        "xt": np.ascontiguousarray(np.asarray(x[b]).T.astype(np.float32)),
        "wqt": np.ascontiguousarray(np.asarray(wq)[sl, :].T.astype(np.float32)),
        "wkt": np.ascontiguousarray(np.asarray(wk)[sl, :].T.astype(np.float32)),
        "wvt": np.ascontiguousarray(np.asarray(wv)[sl, :].T.astype(np.float32)),
        "wot": np.ascontiguousarray(
            np.asarray(wo)[:, sl].T.astype(_bf16())),
    }
    m.update(host_tables(T))
    return m


_CACHE = {}


def _get_nc(T: int = 2048):
    key = ("nc", T)
    if key not in _CACHE:
        nc = bacc.Bacc("TRN2", target_bir_lowering=False, debug=False)
        build_kernel(nc, T)
        nc.compile()
        _CACHE[key] = nc
    return _CACHE[key]


def kernel(x, wq, wk, wv, wo, mask=None):
    from concourse import bass_utils
    nc = _get_nc(2048)
    in_maps = [core_inputs(x, wq, wk, wv, wo, c) for c in range(8)]
    res = bass_utils.run_bass_kernel_spmd(nc, in_maps, list(range(8)))
    outs = [np.asarray(res.results[c]["yt"]) for c in range(8)]
    out = np.empty((4, 2048, 1024), np.float32)
    for b in range(4):
        out[b] = (outs[b] + outs[b + 4]).T
    return out


# revision 30
# speedup vs baseline: 1.9529x; 1.0141x over previous
"""Trainium2 Bass kernel: MultiHeadAttention with QK-RMSNorm + partial rotary,
causal softmax. B=4, T=2048, D=1024, H=16, HD=64, fp32.

Sharding: 8 cores = 4 batches x 2 head-groups (8 heads each). Each core:
  - QKV projections for its batch, restricted to its 512 head-dims
  - causal attention for its 8 heads
  - partial output projection (its 512 contraction dims, all 1024 outputs)
Host sums the two head-group partials per batch (the all-reduce) and
transposes back.

v3 design: phase-interleaved, engine-specialized, single-exp scores.
  - Per 512-token block tt: QKV projections -> rotary+RMS-norm -> attention
    for i-block tt -> output projection of i-block tt. The ACT-heavy softmax
    and the DMA-heavy output store overlap the PE-heavy projections of the
    next block, keeping the PE dense so the HAM clock gate stays at K=8/8
    (fp32r at low clock is 4x slower).
  - K is normalized exactly like Q (bdiag matmul + sqrt + recip + mul), so
    exp needs no per-head scale and one [128,1024] exp op covers both heads
    of a pair (halves ACT instruction count - ACT is the attention pacer).
  - gpsimd runs ONLY partition_broadcast (softmax denominator): any second
    gpsimd ucode kernel would trigger ~6us IRAM reloads per switch.
  - ACT tables: {Square,Sqrt,Copy} + {Exp,Copy} - two loads per block max.
  - V, softmax weights, attention output and wo in bf16 (same PE rate as
    fp32r, halves SBUF/DVE cost, dodges fp32r small-N penalty on diagonal).
  - causal diagonal blocks compute/exp only the valid column strip; mask is
    an additive -300 on scores pre-exp.
"""

import numpy as np
from contextlib import ExitStack

import concourse.bass as bass
import concourse.tile as tile
import concourse.mybir as mybir
from concourse import bacc

F32 = mybir.dt.float32
BF16 = mybir.dt.bfloat16
MM_DT = mybir.dt.float32r
AF = mybir.ActivationFunctionType

D = 1024   # model dim
DH = 512   # head-group width per core (8 heads x 64)
NH = 8     # heads per core
HD = 64    # head dim
NKC = D // 128   # k-chunks over model dim
EPS = 1e-6


def _r(ap):
    return ap.bitcast(MM_DT)


def build_kernel(nc: bass.Bass, T: int = 2048):
    NTT = T // 512     # 512-wide t/i blocks
    NTS = T // 128     # 128-wide t/j chunks

    xt = nc.dram_tensor("xt", [D, T], F32, kind="ExternalInput").ap()
    wqt = nc.dram_tensor("wqt", [D, DH], F32, kind="ExternalInput").ap()
    wkt = nc.dram_tensor("wkt", [D, DH], F32, kind="ExternalInput").ap()
    wvt = nc.dram_tensor("wvt", [D, DH], F32, kind="ExternalInput").ap()
    wot = nc.dram_tensor("wot", [DH, D], BF16, kind="ExternalInput").ap()
    c2d = nc.dram_tensor("c2", [128, T], F32, kind="ExternalInput").ap()
    s2d = nc.dram_tensor("s2", [128, T], F32, kind="ExternalInput").ap()
    pswapd = nc.dram_tensor("pswap", [128, 128], F32, kind="ExternalInput").ap()
    bdiagd = nc.dram_tensor("bdiag", [128, 128], F32, kind="ExternalInput").ap()
    trid = nc.dram_tensor("trimask", [128, 256], F32, kind="ExternalInput").ap()
    yt = nc.dram_tensor("yt", [D, T], BF16, kind="ExternalOutput").ap()

    with tile.TileContext(nc) as tc, ExitStack() as ctx:
        # ---- persistent SBUF pools --------------------------------------
        qk_pool = ctx.enter_context(tc.tile_pool(name="qk", bufs=1))
        v_pool = ctx.enter_context(tc.tile_pool(name="v", bufs=1))
        ot_pool = ctx.enter_context(tc.tile_pool(name="otf", bufs=1))
        const_pool = ctx.enter_context(tc.tile_pool(name="const", bufs=1))
        w_pool = ctx.enter_context(tc.tile_pool(name="wqkv", bufs=1))
        wo_pool = ctx.enter_context(tc.tile_pool(name="wo", bufs=1))
        x_pool = ctx.enter_context(tc.tile_pool(name="xs", bufs=8))
        cs_pool = ctx.enter_context(tc.tile_pool(name="cs", bufs=1))
        t_pool = ctx.enter_context(tc.tile_pool(name="rott", bufs=2))
        p_pool = ctx.enter_context(tc.tile_pool(name="pexp", bufs=3))
        e_pool = ctx.enter_context(tc.tile_pool(name="epi", bufs=1))
        st_pool = ctx.enter_context(tc.tile_pool(name="stg3", bufs=2))
        # single PSUM pool; tags share the 8 banks across phases:
        #   A: proj accum [128,512] + scores pair [128,1024]   (2x2 banks)
        #   B: rotary pswap out + attn AV accum                (2 banks)
        #   C: rotary bdiag out + out-proj accum               (2 banks)
        ps = ctx.enter_context(tc.tile_pool(name="ps", bufs=1, space="PSUM"))

        qt_s = [qk_pool.tile([128, T], F32, name=f"qt{j}") for j in range(4)]
        kt_s = [qk_pool.tile([128, T], F32, name=f"kt{j}") for j in range(4)]
        v_s = [v_pool.tile([128, NH * 65], BF16, name=f"vt{j}")
               for j in range(NTS)]
        otf = [ot_pool.tile([128, T], BF16, name=f"otf{j}") for j in range(4)]

        # weights first: they gate the first projection matmuls.
        wq_s = [w_pool.tile([128, DH], F32, name=f"wq{k}") for k in range(NKC)]
        wk_s = [w_pool.tile([128, DH], F32, name=f"wk{k}") for k in range(NKC)]
        wv_s = [w_pool.tile([128, DH], F32, name=f"wv{k}") for k in range(NKC)]
        for k in range(NKC):
            ksl = slice(k * 128, (k + 1) * 128)
            nc.sync.dma_start(_r(wq_s[k][:]), _r(wqt[ksl, :]))
        pswap = const_pool.tile([128, 128], F32, name="pswap_s")
        bdiag = const_pool.tile([128, 128], F32, name="bdiag_s")
        trif = const_pool.tile([128, 256], F32, name="trif_s")
        nc.scalar.dma_start(_r(pswap[:]), _r(pswapd[:]))
        nc.scalar.dma_start(_r(bdiag[:]), _r(bdiagd[:]))
        for k in range(NKC):
            ksl = slice(k * 128, (k + 1) * 128)
            nc.scalar.dma_start(_r(wk_s[k][:]), _r(wkt[ksl, :]))
            nc.scalar.dma_start(_r(wv_s[k][:]), _r(wvt[ksl, :]))
        nc.scalar.dma_start(trif[:], trid[:])
        wob = [wo_pool.tile([128, D], BF16, name=f"wob{k}") for k in range(4)]
        for k4 in range(4):
            nc.scalar.dma_start(wob[k4][:], wot[k4 * 128:(k4 + 1) * 128, :])
        epsb = const_pool.tile([128, 1], F32, name="epsb")
        nc.vector.memset(epsb[:], 8.0 * EPS)
        onescb = const_pool.tile([128, NH], BF16, name="onescb")
        nc.vector.memset(onescb[:], 1.0)

        def py_chain(tt, pib, dt_):
            """Out-projection of one dout chunk of i-block pib (PE filler)."""
            dsl = slice(dt_ * 128, (dt_ + 1) * 128)
            psl = slice(pib * 512, (pib + 1) * 512)
            py = ps.tile([128, 512], F32, name="py", tag="C", bufs=2)
            for k4 in range(4):
                nc.tensor.matmul(py[:], wob[k4][:, dsl], otf[k4][:, psl],
                                 start=(k4 == 0), stop=(k4 == 3))
            st = st_pool.tile([128, 512], BF16, name="st", tag="st", bufs=4)
            if tt < 2:
                nc.scalar.copy(st[:], py[:])
            else:
                nc.vector.tensor_copy(st[:], py[:])
            nc.sync.dma_start(yt[dsl, psl], st[:])

        for tt in range(NTT):
            tsl = slice(tt * 512, (tt + 1) * 512)
            early = tt < 2      # ACT has slack early, DVE late

            def aux_copy(dst_ap, src_ap):
                if early:
                    nc.scalar.copy(dst_ap, src_ap)
                else:
                    nc.vector.tensor_copy(dst_ap, src_ap)

            # x chunks + rotary tables for this block
            xts = []
            for k in range(NKC):
                xc = x_pool.tile([128, 512], F32, name="xc", tag="xc")
                nc.gpsimd.dma_start(_r(xc[:]),
                                    _r(xt[k * 128:(k + 1) * 128, tsl]))
                xts.append(xc)
            c2t = cs_pool.tile([128, 512], F32, name="c2t", tag="c2t")
            s2t = cs_pool.tile([128, 512], F32, name="s2t", tag="s2t")
            nc.sync.dma_start(c2t[:], c2d[:, tsl])
            nc.sync.dma_start(s2t[:], s2d[:, tsl])

            # ---- projections: PE stays dense, copies trail -------------
            for (wsrc, nm) in ((wq_s, "q"), (wk_s, "k")):
                for hp in range(4):
                    jsl = slice(hp * 128, (hp + 1) * 128)
                    pp = ps.tile([128, 1024], F32, name="pp", tag="A", bufs=2)
                    for k in range(NKC):
                        nc.tensor.matmul(pp[:, 0:512], _r(wsrc[k][:, jsl]),
                                         _r(xts[k][:]),
                                         start=(k == 0), stop=(k == NKC - 1))
                    dst = (qt_s if nm == "q" else kt_s)[hp]
                    aux_copy(_r(dst[:, tsl]), pp[:, 0:512])
            for ts_ in range(4):
                ci = tt * 4 + ts_
                pv = ps.tile([128, 1024], F32, name="pv", tag="A", bufs=2)
                for k in range(NKC):
                    nc.tensor.matmul(
                        pv[:, 0:512], _r(xts[k][:, ts_ * 128:(ts_ + 1) * 128]),
                        _r(wv_s[k][:]),
                        start=(k == 0), stop=(k == NKC - 1))
                v3 = v_s[ci].rearrange("p (h e) -> p h e", h=NH)
                aux_copy(v3[:, :, 0:64],
                         pv[:, 0:512].rearrange("p (h e) -> p h e", h=NH))
                nc.vector.tensor_copy(v3[:, :, 64:65], onescb[:].unsqueeze(-1))

            # ---- rotary + RMS-norm for q and k (identical paths) -------
            # rsqrt = exp(-0.5*ln(x)): Square/Ln/Exp/Copy share ONE ACT
            # table, so the whole kernel pays a single ACT_TABLE_LOAD.
            for hp in range(4):
                for nm in ("q", "k"):
                    dst = (qt_s if nm == "q" else kt_s)[hp]
                    sq = t_pool.tile([128, 512], F32, name="sq", tag=f"sq{nm}")
                    if early:
                        nc.scalar.activation(_r(sq[:]), dst[:, tsl], AF.Square)
                    else:
                        nc.vector.tensor_mul(_r(sq[:]), dst[:, tsl],
                                             dst[:, tsl])
                    xs_ = ps.tile([128, 512], F32, name="xs", tag="B", bufs=2)
                    nc.tensor.matmul(xs_[:], _r(pswap[:]), _r(dst[:, tsl]),
                                     start=True, stop=True)
                    ms = ps.tile([128, 512], F32, name="ms", tag="C", bufs=2)
                    nc.tensor.matmul(ms[:], _r(bdiag[:]), _r(sq[:]),
                                     start=True, stop=True)
                    s1 = t_pool.tile([128, 512], F32, name="s1", tag=f"s1{nm}")
                    nc.scalar.activation(s1[:], ms[:], AF.Ln,
                                         scale=0.125, bias=epsb[:])
                    nc.scalar.activation(s1[:], s1[:], AF.Exp, scale=-0.5)
                    nc.vector.tensor_mul(_r(dst[:, tsl]), dst[:, tsl],
                                         c2t[:])
                    nc.vector.tensor_mul(xs_[:], xs_[:], s2t[:])
                    nc.vector.tensor_add(_r(dst[:, tsl]), dst[:, tsl], xs_[:])
                    nc.vector.tensor_mul(_r(dst[:, tsl]), dst[:, tsl], s1[:])

            # ================= attention for i-block ib = tt ============
            # Software-pipelined: scores(jt+1) is emitted before AV(jt) so
            # the PE streams through exp's latency; the previous block's
            # out-projection chains fill PE time while epilogues drain.
            ib = tt
            isl = tsl
            nj = 4 * ib + 4
            fillers = [(tt - 1, d) for d in range(8)] if tt > 0 else []

            def emit_sc(hp, jt):
                jsl = slice(jt * 128, (jt + 1) * 128)
                c_ = jt - 4 * ib          # >=0 on diagonal chunks
                off = 128 * c_ if c_ >= 0 else 0
                osc = off if off <= 256 else 256   # keep f32r N>=256
                sc = ps.tile([128, 1024], F32, name="sc", tag="A", bufs=2)
                for h2 in range(2):
                    ho = h2 * 64
                    nc.tensor.matmul(
                        sc[:, 512 * h2 + osc:512 * h2 + 512],
                        _r(kt_s[hp][ho:ho + 64, jsl]),
                        _r(qt_s[hp][ho:ho + 64,
                                    ib * 512 + osc:ib * 512 + 512]),
                        start=True, stop=True)
                return sc, off, c_

            for hp in range(4):
                ot_ps = [ps.tile([128, 512], F32, name="otp", tag="B", bufs=2)
                         for _ in range(2)]
                nxt = emit_sc(hp, 0)
                for jt in range(nj):
                    sc, off, c_ = nxt
                    sc3 = sc.rearrange("p (h e) -> p h e", h=2)
                    if c_ >= 0:
                        # additive causal mask (-300 below diag) pre-exp
                        nc.vector.tensor_add(
                            sc3[:, :, off:off + 128], sc3[:, :, off:off + 128],
                            trif[:].rearrange("p (h e) -> p h e", h=2))
                    p = p_pool.tile([128, 1024], BF16, name="p", tag="p")
                    p3 = p.rearrange("p (h e) -> p h e", h=2)
                    nc.scalar.activation(p3[:, :, off:512], sc3[:, :, off:512],
                                         AF.Exp)
                    if jt + 1 < nj:
                        nxt = emit_sc(hp, jt + 1)
                    for h2 in range(2):
                        h = 2 * hp + h2
                        nc.tensor.matmul(
                            ot_ps[h2][0:65, off:512],
                            v_s[jt][:, 65 * h:65 * h + 65],
                            p[:, 512 * h2 + off:512 * h2 + 512],
                            start=(jt == 0), stop=(jt == nj - 1))
                # epilogue: divide by the denominator row (psum row 64).
                # partition_broadcast is the ONLY gpsimd ucode kernel here,
                # so its IRAM load is paid once.
                for h2 in range(2):
                    op = ot_ps[h2]
                    den = e_pool.tile([1, 512], F32, name="den", tag="den")
                    aux_copy(den[:], op[64:65, :])
                    rden = e_pool.tile([1, 512], F32, name="rden", tag="rden")
                    nc.vector.reciprocal_approx_fast(out=rden[:], in_=den[:])
                    rb = e_pool.tile([64, 512], F32, name="rb", tag="rb")
                    nc.gpsimd.partition_broadcast(rb[:], rden[:], channels=64)
                    ho = h2 * 64
                    nc.vector.tensor_mul(otf[hp][ho:ho + 64, isl],
                                         op[0:64, :], rb[:])
                for _ in range(2):
                    if fillers:
                        pib, d = fillers.pop(0)
                        py_chain(tt, pib, d)

        # out-projection of the final i-block (nothing left to hide it)
        for dt_ in range(8):
            py_chain(NTT - 1, NTT - 1, dt_)
    return nc


# ---------------- host-side tables & shard prep -------------------------

def host_tables(T: int = 2048):
    n = HD // 4
    af = (1.0 / 1024) ** np.linspace(0, 1, n, dtype=np.float32)
    af = np.concatenate([af, np.zeros(n, np.float32)])  # [32]
    theta = np.outer(np.arange(T, dtype=np.float32), af)  # [T, 32]
    cosT = np.cos(theta).T.astype(np.float32)  # [32, T]
    sinT = np.sin(theta).T.astype(np.float32)
    c2 = np.tile(cosT, (4, 1))                             # [128, T]
    s2 = np.tile(np.concatenate([sinT, -sinT], 0), (2, 1))  # [128, T]
    km = np.arange(128)
    pswap = (km[:, None] == (km[None, :] ^ 32)).astype(np.float32)
    bdiag = ((km[:, None] // 64) == (km[None, :] // 64)).astype(np.float32)
    r_ = np.arange(128)[:, None]
    c_ = np.arange(128)[None, :]
    tri1 = np.where(c_ >= r_, 0.0, -300.0).astype(np.float32)
    tri = np.concatenate([tri1, tri1], axis=1)             # [128, 256]
    return {"c2": np.ascontiguousarray(c2), "s2": np.ascontiguousarray(s2),
            "pswap": pswap, "bdiag": bdiag,
            "trimask": np.ascontiguousarray(tri)}


def _bf16():
    import ml_dtypes
    return ml_dtypes.bfloat16


def core_inputs(x, wq, wk, wv, wo, core: int, T: int = 2048):
    b, g = core % 4, core // 4
    sl = slice(g * DH, (g + 1) * DH)
    m = {
        "xt": np.ascontiguousarray(np.asarray(x[b]).T.astype(np.float32)),
        "wqt": np.ascontiguousarray(np.asarray(wq)[sl, :].T.astype(np.float32)),
        "wkt": np.ascontiguousarray(np.asarray(wk)[sl, :].T.astype(np.float32)),
        "wvt": np.ascontiguousarray(np.asarray(wv)[sl, :].T.astype(np.float32)),
        "wot": np.ascontiguousarray(
            np.asarray(wo)[:, sl].T.astype(_bf16())),
    }
    m.update(host_tables(T))
    return m


_CACHE = {}


def _get_nc(T: int = 2048):
    key = ("nc", T)
    if key not in _CACHE:
        nc = bacc.Bacc("TRN2", target_bir_lowering=False, debug=False)
        build_kernel(nc, T)
        nc.compile()
        _CACHE[key] = nc
    return _CACHE[key]


def kernel(x, wq, wk, wv, wo, mask=None):
    from concourse import bass_utils
    nc = _get_nc(2048)
    in_maps = [core_inputs(x, wq, wk, wv, wo, c) for c in range(8)]
    res = bass_utils.run_bass_kernel_spmd(nc, in_maps, list(range(8)))
    outs = [np.asarray(res.results[c]["yt"]).astype(np.float32)
            for c in range(8)]
    out = np.empty((4, 2048, 1024), np.float32)
    for b in range(4):
        out[b] = (outs[b] + outs[b + 4]).T
    return out


# revision 31
# speedup vs baseline: 1.9696x; 1.0085x over previous
"""Trainium2 Bass kernel: MultiHeadAttention with QK-RMSNorm + partial rotary,
causal softmax. B=4, T=2048, D=1024, H=16, HD=64, fp32.

Sharding: 8 cores = 4 batches x 2 head-groups (8 heads each). Each core:
  - QKV projections for its batch, restricted to its 512 head-dims
  - causal attention for its 8 heads
  - partial output projection (its 512 contraction dims, all 1024 outputs)
Host sums the two head-group partials per batch (the all-reduce) and
transposes back.

v3 design: phase-interleaved, engine-specialized, single-exp scores.
  - Per 512-token block tt: QKV projections -> rotary+RMS-norm -> attention
    for i-block tt -> output projection of i-block tt. The ACT-heavy softmax
    and the DMA-heavy output store overlap the PE-heavy projections of the
    next block, keeping the PE dense so the HAM clock gate stays at K=8/8
    (fp32r at low clock is 4x slower).
  - K is normalized exactly like Q (bdiag matmul + sqrt + recip + mul), so
    exp needs no per-head scale and one [128,1024] exp op covers both heads
    of a pair (halves ACT instruction count - ACT is the attention pacer).
  - gpsimd runs ONLY partition_broadcast (softmax denominator): any second
    gpsimd ucode kernel would trigger ~6us IRAM reloads per switch.
  - ACT tables: {Square,Sqrt,Copy} + {Exp,Copy} - two loads per block max.
  - V, softmax weights, attention output and wo in bf16 (same PE rate as
    fp32r, halves SBUF/DVE cost, dodges fp32r small-N penalty on diagonal).
  - causal diagonal blocks compute/exp only the valid column strip; mask is
    an additive -300 on scores pre-exp.
"""

import numpy as np
from contextlib import ExitStack

import concourse.bass as bass
import concourse.tile as tile
import concourse.mybir as mybir
from concourse import bacc

F32 = mybir.dt.float32
BF16 = mybir.dt.bfloat16
MM_DT = mybir.dt.float32r
AF = mybir.ActivationFunctionType

D = 1024   # model dim
DH = 512   # head-group width per core (8 heads x 64)
NH = 8     # heads per core
HD = 64    # head dim
NKC = D // 128   # k-chunks over model dim
EPS = 1e-6


def _r(ap):
    return ap.bitcast(MM_DT)


def build_kernel(nc: bass.Bass, T: int = 2048):
    NTT = T // 512     # 512-wide t/i blocks
    NTS = T // 128     # 128-wide t/j chunks

    xt = nc.dram_tensor("xt", [D, T], F32, kind="ExternalInput").ap()
    wqt = nc.dram_tensor("wqt", [D, DH], F32, kind="ExternalInput").ap()
    wkt = nc.dram_tensor("wkt", [D, DH], F32, kind="ExternalInput").ap()
    wvt = nc.dram_tensor("wvt", [D, DH], F32, kind="ExternalInput").ap()
    wot = nc.dram_tensor("wot", [DH, D], BF16, kind="ExternalInput").ap()
    c2d = nc.dram_tensor("c2", [128, T], F32, kind="ExternalInput").ap()
    s2d = nc.dram_tensor("s2", [128, T], F32, kind="ExternalInput").ap()
    pswapd = nc.dram_tensor("pswap", [128, 128], F32, kind="ExternalInput").ap()
    bdiagd = nc.dram_tensor("bdiag", [128, 128], F32, kind="ExternalInput").ap()
    trid = nc.dram_tensor("trimask", [128, 256], F32, kind="ExternalInput").ap()
    yt = nc.dram_tensor("yt", [D, T], BF16, kind="ExternalOutput").ap()

    with tile.TileContext(nc) as tc, ExitStack() as ctx:
        # ---- persistent SBUF pools --------------------------------------
        qk_pool = ctx.enter_context(tc.tile_pool(name="qk", bufs=1))
        v_pool = ctx.enter_context(tc.tile_pool(name="v", bufs=1))
        ot_pool = ctx.enter_context(tc.tile_pool(name="otf", bufs=1))
        const_pool = ctx.enter_context(tc.tile_pool(name="const", bufs=1))
        w_pool = ctx.enter_context(tc.tile_pool(name="wqkv", bufs=1))
        wo_pool = ctx.enter_context(tc.tile_pool(name="wo", bufs=1))
        x_pool = ctx.enter_context(tc.tile_pool(name="xs", bufs=8))
        cs_pool = ctx.enter_context(tc.tile_pool(name="cs", bufs=1))
        t_pool = ctx.enter_context(tc.tile_pool(name="rott", bufs=2))
        p_pool = ctx.enter_context(tc.tile_pool(name="pexp", bufs=3))
        e_pool = ctx.enter_context(tc.tile_pool(name="epi", bufs=1))
        st_pool = ctx.enter_context(tc.tile_pool(name="stg3", bufs=2))
        # single PSUM pool; tags share the 8 banks across phases:
        #   A: proj accum [128,512] + scores pair [128,1024]   (2x2 banks)
        #   B: rotary pswap out + attn AV accum                (2 banks)
        #   C: rotary bdiag out + out-proj accum               (2 banks)
        ps = ctx.enter_context(tc.tile_pool(name="ps", bufs=1, space="PSUM"))

        qt_s = [qk_pool.tile([128, T], F32, name=f"qt{j}") for j in range(4)]
        kt_s = [qk_pool.tile([128, T], F32, name=f"kt{j}") for j in range(4)]
        v_s = [v_pool.tile([128, NH * 65], BF16, name=f"vt{j}")
               for j in range(NTS)]
        otf = [ot_pool.tile([128, T], BF16, name=f"otf{j}") for j in range(4)]

        # weights first: they gate the first projection matmuls.
        wq_s = [w_pool.tile([128, DH], F32, name=f"wq{k}") for k in range(NKC)]
        wk_s = [w_pool.tile([128, DH], F32, name=f"wk{k}") for k in range(NKC)]
        wv_s = [w_pool.tile([128, DH], F32, name=f"wv{k}") for k in range(NKC)]
        for k in range(NKC):
            ksl = slice(k * 128, (k + 1) * 128)
            nc.sync.dma_start(_r(wq_s[k][:]), _r(wqt[ksl, :]))
        pswap = const_pool.tile([128, 128], F32, name="pswap_s")
        bdiag = const_pool.tile([128, 128], F32, name="bdiag_s")
        trif = const_pool.tile([128, 256], F32, name="trif_s")
        nc.scalar.dma_start(_r(pswap[:]), _r(pswapd[:]))
        nc.scalar.dma_start(_r(bdiag[:]), _r(bdiagd[:]))
        for k in range(NKC):
            ksl = slice(k * 128, (k + 1) * 128)
            nc.scalar.dma_start(_r(wk_s[k][:]), _r(wkt[ksl, :]))
            nc.scalar.dma_start(_r(wv_s[k][:]), _r(wvt[ksl, :]))
        nc.scalar.dma_start(trif[:], trid[:])
        wob = [wo_pool.tile([128, D], BF16, name=f"wob{k}") for k in range(4)]
        for k4 in range(4):
            nc.scalar.dma_start(wob[k4][:], wot[k4 * 128:(k4 + 1) * 128, :])
        epsb = const_pool.tile([128, 1], F32, name="epsb")
        nc.vector.memset(epsb[:], 8.0 * EPS)
        onescb = const_pool.tile([128, NH], BF16, name="onescb")
        nc.vector.memset(onescb[:], 1.0)

        # ---------------- emission helpers ------------------------------
        def emit_x_dma(tt):
            tsl = slice(tt * 512, (tt + 1) * 512)
            xts = []
            for k in range(NKC):
                xc = x_pool.tile([128, 512], F32, name="xc", tag="xc")
                nc.gpsimd.dma_start(_r(xc[:]),
                                    _r(xt[k * 128:(k + 1) * 128, tsl]))
                xts.append(xc)
            return xts

        def py_chain(pib, dt_):
            """Out-projection of one dout chunk of i-block pib (PE filler)."""
            dsl = slice(dt_ * 128, (dt_ + 1) * 128)
            psl = slice(pib * 512, (pib + 1) * 512)
            py = ps.tile([128, 512], F32, name="py", tag="C", bufs=2)
            for k4 in range(4):
                nc.tensor.matmul(py[:], wob[k4][:, dsl], otf[k4][:, psl],
                                 start=(k4 == 0), stop=(k4 == 3))
            st = st_pool.tile([128, 512], BF16, name="st", tag="st", bufs=4)
            nc.vector.tensor_copy(st[:], py[:])
            nc.sync.dma_start(yt[dsl, psl], st[:])

        def proj_closures(tt, xts):
            """12 PE projection k-groups for block tt (q/k/v); ACT copies
            trail. Interleavable with the previous block's attention."""
            tsl = slice(tt * 512, (tt + 1) * 512)
            out = []
            for (wsrc, dsts) in ((wq_s, qt_s), (wk_s, kt_s)):
                for hp in range(4):
                    def g(wsrc=wsrc, dsts=dsts, hp=hp):
                        jsl = slice(hp * 128, (hp + 1) * 128)
                        pp = ps.tile([128, 1024], F32, name="pp", tag="A",
                                     bufs=2)
                        for k in range(NKC):
                            nc.tensor.matmul(
                                pp[:, 0:512], _r(wsrc[k][:, jsl]),
                                _r(xts[k][:]),
                                start=(k == 0), stop=(k == NKC - 1))
                        nc.scalar.copy(_r(dsts[hp][:, tsl]), pp[:, 0:512])
                    out.append(g)
            for ts_ in range(4):
                def g(ts_=ts_):
                    ci = tt * 4 + ts_
                    pv = ps.tile([128, 1024], F32, name="pv", tag="A", bufs=2)
                    for k in range(NKC):
                        nc.tensor.matmul(
                            pv[:, 0:512],
                            _r(xts[k][:, ts_ * 128:(ts_ + 1) * 128]),
                            _r(wv_s[k][:]),
                            start=(k == 0), stop=(k == NKC - 1))
                    v3 = v_s[ci].rearrange("p (h e) -> p h e", h=NH)
                    nc.scalar.copy(
                        v3[:, :, 0:64],
                        pv[:, 0:512].rearrange("p (h e) -> p h e", h=NH))
                    nc.vector.tensor_copy(v3[:, :, 64:65],
                                          onescb[:].unsqueeze(-1))
                out.append(g)
            return out

        def rot_closures(tt):
            """Rotary + RMS-norm for q and k of block tt. Squares on DVE,
            sqrt on ACT (Square/Sqrt/Copy + Exp/Copy = 2 tables; squares
            and sqrts batch per block so table swaps stay ~2/block)."""
            tsl = slice(tt * 512, (tt + 1) * 512)
            c2t = cs_pool.tile([128, 512], F32, name="c2t", tag="c2t")
            s2t = cs_pool.tile([128, 512], F32, name="s2t", tag="s2t")
            nc.sync.dma_start(c2t[:], c2d[:, tsl])
            nc.sync.dma_start(s2t[:], s2d[:, tsl])
            out = []
            for hp in range(4):
                for nm in ("q", "k"):
                    def g(hp=hp, nm=nm):
                        dst = (qt_s if nm == "q" else kt_s)[hp]
                        sq = t_pool.tile([128, 512], F32, name="sq",
                                         tag=f"sq{nm}")
                        nc.vector.scalar_tensor_tensor(
                            _r(sq[:]), dst[:, tsl], 1.0, dst[:, tsl],
                            mybir.AluOpType.mult, mybir.AluOpType.mult)
                        xs_ = ps.tile([128, 512], F32, name="xs", tag="B",
                                      bufs=2)
                        nc.tensor.matmul(xs_[:], _r(pswap[:]),
                                         _r(dst[:, tsl]),
                                         start=True, stop=True)
                        ms = ps.tile([128, 512], F32, name="ms", tag="C",
                                     bufs=2)
                        nc.tensor.matmul(ms[:], _r(bdiag[:]), _r(sq[:]),
                                         start=True, stop=True)
                        s1 = t_pool.tile([128, 512], F32, name="s1",
                                         tag=f"s1{nm}")
                        nc.scalar.activation(s1[:], ms[:], AF.Sqrt,
                                             scale=0.125, bias=epsb[:])
                        nc.vector.reciprocal_approx_fast(out=s1[:], in_=s1[:])
                        nc.vector.tensor_mul(_r(dst[:, tsl]), dst[:, tsl],
                                             c2t[:])
                        nc.vector.tensor_mul(xs_[:], xs_[:], s2t[:])
                        nc.vector.tensor_add(_r(dst[:, tsl]), dst[:, tsl],
                                             xs_[:])
                        nc.vector.tensor_mul(_r(dst[:, tsl]), dst[:, tsl],
                                             s1[:])
                    out.append(g)
            return out

        def attn_closures(tt):
            """Software-pipelined attention for i-block tt: scores(jt+1) is
            emitted before AV(jt) so interleaved PE work covers exp."""
            ib = tt
            isl = slice(tt * 512, (tt + 1) * 512)
            nj = 4 * ib + 4
            late = tt >= 2

            def emit_sc(hp, jt):
                jsl = slice(jt * 128, (jt + 1) * 128)
                c_ = jt - 4 * ib          # >=0 on diagonal chunks
                off = 128 * c_ if c_ >= 0 else 0
                osc = off if off <= 256 else 256   # keep f32r N>=256
                sc = ps.tile([128, 1024], F32, name="sc", tag="A", bufs=2)
                for h2 in range(2):
                    ho = h2 * 64
                    nc.tensor.matmul(
                        sc[:, 512 * h2 + osc:512 * h2 + 512],
                        _r(kt_s[hp][ho:ho + 64, jsl]),
                        _r(qt_s[hp][ho:ho + 64,
                                    ib * 512 + osc:ib * 512 + 512]),
                        start=True, stop=True)
                return sc, off, c_

            out = []
            for hp in range(4):
                box = {}

                def c_start(hp=hp, box=box):
                    box["ot"] = [ps.tile([128, 512], F32, name="otp",
                                         tag="B", bufs=2) for _ in range(2)]
                    box["nxt"] = emit_sc(hp, 0)
                out.append(c_start)

                for jt in range(nj):
                    def c_item(hp=hp, jt=jt, box=box):
                        sc, off, c_ = box["nxt"]
                        sc3 = sc.rearrange("p (h e) -> p h e", h=2)
                        if c_ >= 0:
                            # additive causal mask (-300 below diag) pre-exp
                            nc.vector.tensor_add(
                                sc3[:, :, off:off + 128],
                                sc3[:, :, off:off + 128],
                                trif[:].rearrange("p (h e) -> p h e", h=2))
                        p = p_pool.tile([128, 1024], BF16, name="p", tag="p")
                        p3 = p.rearrange("p (h e) -> p h e", h=2)
                        nc.scalar.activation(p3[:, :, off:512],
                                             sc3[:, :, off:512], AF.Exp)
                        if jt + 1 < nj:
                            box["nxt"] = emit_sc(hp, jt + 1)
                        for h2 in range(2):
                            h = 2 * hp + h2
                            nc.tensor.matmul(
                                box["ot"][h2][0:65, off:512],
                                v_s[jt][:, 65 * h:65 * h + 65],
                                p[:, 512 * h2 + off:512 * h2 + 512],
                                start=(jt == 0), stop=(jt == nj - 1))
                    out.append(c_item)

                def c_epi(hp=hp, box=box):
                    # epilogue: divide by the denominator row (psum row 64).
                    # partition_broadcast is the ONLY gpsimd ucode kernel in
                    # the program, so its IRAM load is paid once.
                    for h2 in range(2):
                        op = box["ot"][h2]
                        den = e_pool.tile([1, 512], F32, name="den",
                                          tag="den")
                        if late:
                            nc.vector.tensor_copy(den[:], op[64:65, :])
                        else:
                            nc.scalar.copy(den[:], op[64:65, :])
                        rden = e_pool.tile([1, 512], F32, name="rden",
                                           tag="rden")
                        nc.vector.reciprocal_approx_fast(out=rden[:],
                                                         in_=den[:])
                        rb = e_pool.tile([64, 512], F32, name="rb", tag="rb")
                        nc.gpsimd.partition_broadcast(rb[:], rden[:],
                                                      channels=64)
                        ho = h2 * 64
                        nc.vector.tensor_mul(otf[hp][ho:ho + 64, isl],
                                             op[0:64, :], rb[:])
                out.append(c_epi)
            return out

        def interleave(a, b):
            ia = ib_ = 0
            while ia < len(a) or ib_ < len(b):
                if ib_ >= len(b) or (ia < len(a)
                                     and ia * len(b) <= ib_ * len(a)):
                    a[ia]()
                    ia += 1
                else:
                    b[ib_]()
                    ib_ += 1

        # ---------------- schedule --------------------------------------
        xts = emit_x_dma(0)
        for f in proj_closures(0, xts):
            f()
        pending_py = []
        for tt in range(NTT):
            a_items = rot_closures(tt) + attn_closures(tt)
            b_items = []
            if tt + 1 < NTT:
                xts = emit_x_dma(tt + 1)
                b_items += proj_closures(tt + 1, xts)
            if tt == NTT - 1:
                take = [p_ for p_ in pending_py if p_[0] <= tt - 1]
            else:
                take = [p_ for p_ in pending_py if p_[0] == tt - 2]
            for p_ in take:
                pending_py.remove(p_)
                b_items.append(lambda p_=p_: py_chain(*p_))
            interleave(a_items, b_items)
            pending_py += [(tt, d) for d in range(8)]
        for pib, d in pending_py:
            py_chain(pib, d)
    return nc


# ---------------- host-side tables & shard prep -------------------------

def host_tables(T: int = 2048):
    n = HD // 4
    af = (1.0 / 1024) ** np.linspace(0, 1, n, dtype=np.float32)
    af = np.concatenate([af, np.zeros(n, np.float32)])  # [32]
    theta = np.outer(np.arange(T, dtype=np.float32), af)  # [T, 32]
    cosT = np.cos(theta).T.astype(np.float32)  # [32, T]
    sinT = np.sin(theta).T.astype(np.float32)
    c2 = np.tile(cosT, (4, 1))                             # [128, T]
    s2 = np.tile(np.concatenate([sinT, -sinT], 0), (2, 1))  # [128, T]
    km = np.arange(128)
    pswap = (km[:, None] == (km[None, :] ^ 32)).astype(np.float32)
    bdiag = ((km[:, None] // 64) == (km[None, :] // 64)).astype(np.float32)
    r_ = np.arange(128)[:, None]
    c_ = np.arange(128)[None, :]
    tri1 = np.where(c_ >= r_, 0.0, -300.0).astype(np.float32)
    tri = np.concatenate([tri1, tri1], axis=1)             # [128, 256]
    return {"c2": np.ascontiguousarray(c2), "s2": np.ascontiguousarray(s2),
            "pswap": pswap, "bdiag": bdiag,
            "trimask": np.ascontiguousarray(tri)}


def _bf16():
    import ml_dtypes
    return ml_dtypes.bfloat16


def core_inputs(x, wq, wk, wv, wo, core: int, T: int = 2048):
    b, g = core % 4, core // 4
    sl = slice(g * DH, (g + 1) * DH)
    m = {
        "xt": np.ascontiguousarray(np.asarray(x[b]).T.astype(np.float32)),
        "wqt": np.ascontiguousarray(np.asarray(wq)[sl, :].T.astype(np.float32)),
        "wkt": np.ascontiguousarray(np.asarray(wk)[sl, :].T.astype(np.float32)),
        "wvt": np.ascontiguousarray(np.asarray(wv)[sl, :].T.astype(np.float32)),
        "wot": np.ascontiguousarray(
            np.asarray(wo)[:, sl].T.astype(_bf16())),
    }
    m.update(host_tables(T))
    return m


_CACHE = {}


def _get_nc(T: int = 2048):
    key = ("nc", T)
    if key not in _CACHE:
        nc = bacc.Bacc("TRN2", target_bir_lowering=False, debug=False)
        build_kernel(nc, T)
        nc.compile()
        _CACHE[key] = nc
    return _CACHE[key]


def kernel(x, wq, wk, wv, wo, mask=None):
    from concourse import bass_utils
    nc = _get_nc(2048)
    in_maps = [core_inputs(x, wq, wk, wv, wo, c) for c in range(8)]
    res = bass_utils.run_bass_kernel_spmd(nc, in_maps, list(range(8)))
    outs = [np.asarray(res.results[c]["yt"]).astype(np.float32)
            for c in range(8)]
    out = np.empty((4, 2048, 1024), np.float32)
    for b in range(4):
        out[b] = (outs[b] + outs[b + 4]).T
    return out


# revision 32
# speedup vs baseline: 2.0941x; 1.0632x over previous
"""Trainium2 Bass kernel: MultiHeadAttention with QK-RMSNorm + partial rotary,
causal softmax. B=4, T=2048, D=1024, H=16, HD=64, fp32.

Sharding: 8 cores = 4 batches x 2 head-groups (8 heads each). Each core:
  - QKV projections for its batch, restricted to its 512 head-dims
  - causal attention for its 8 heads
  - partial output projection (its 512 contraction dims, all 1024 outputs)
Host sums the two head-group partials per batch (the all-reduce) and
transposes back.

v3 design: phase-interleaved, engine-specialized, single-exp scores.
  - Per 512-token block tt: QKV projections -> rotary+RMS-norm -> attention
    for i-block tt -> output projection of i-block tt. The ACT-heavy softmax
    and the DMA-heavy output store overlap the PE-heavy projections of the
    next block, keeping the PE dense so the HAM clock gate stays at K=8/8
    (fp32r at low clock is 4x slower).
  - K is normalized exactly like Q (bdiag matmul + sqrt + recip + mul), so
    exp needs no per-head scale and one [128,1024] exp op covers both heads
    of a pair (halves ACT instruction count - ACT is the attention pacer).
  - gpsimd runs ONLY partition_broadcast (softmax denominator): any second
    gpsimd ucode kernel would trigger ~6us IRAM reloads per switch.
  - ACT tables: {Square,Sqrt,Copy} + {Exp,Copy} - two loads per block max.
  - V, softmax weights, attention output and wo in bf16 (same PE rate as
    fp32r, halves SBUF/DVE cost, dodges fp32r small-N penalty on diagonal).
  - causal diagonal blocks compute/exp only the valid column strip; mask is
    an additive -300 on scores pre-exp.
"""

import numpy as np
from contextlib import ExitStack

import concourse.bass as bass
import concourse.tile as tile
import concourse.mybir as mybir
from concourse import bacc

F32 = mybir.dt.float32
BF16 = mybir.dt.bfloat16
MM_DT = mybir.dt.float32r
AF = mybir.ActivationFunctionType

D = 1024   # model dim
DH = 512   # head-group width per core (8 heads x 64)
NH = 8     # heads per core
HD = 64    # head dim
NKC = D // 128   # k-chunks over model dim
EPS = 1e-6


def _r(ap):
    return ap.bitcast(MM_DT)


def build_kernel(nc: bass.Bass, T: int = 2048):
    NTT = T // 512     # 512-wide t/i blocks
    NTS = T // 128     # 128-wide t/j chunks

    xt = nc.dram_tensor("xt", [D, T], F32, kind="ExternalInput").ap()
    wqt = nc.dram_tensor("wqt", [D, DH], F32, kind="ExternalInput").ap()
    wkt = nc.dram_tensor("wkt", [D, DH], F32, kind="ExternalInput").ap()
    wvt = nc.dram_tensor("wvt", [D, DH], F32, kind="ExternalInput").ap()
    wot = nc.dram_tensor("wot", [DH, D], BF16, kind="ExternalInput").ap()
    c2d = nc.dram_tensor("c2", [128, T], F32, kind="ExternalInput").ap()
    s2d = nc.dram_tensor("s2", [128, T], F32, kind="ExternalInput").ap()
    pswapd = nc.dram_tensor("pswap", [128, 128], F32, kind="ExternalInput").ap()
    bdiagd = nc.dram_tensor("bdiag", [128, 128], F32, kind="ExternalInput").ap()
    trid = nc.dram_tensor("trimask", [128, 256], F32, kind="ExternalInput").ap()
    yt = nc.dram_tensor("yt", [D, T], BF16, kind="ExternalOutput").ap()

    with tile.TileContext(nc) as tc, ExitStack() as ctx:
        # ---- persistent SBUF pools --------------------------------------
        qk_pool = ctx.enter_context(tc.tile_pool(name="qk", bufs=1))
        v_pool = ctx.enter_context(tc.tile_pool(name="v", bufs=1))
        ot_pool = ctx.enter_context(tc.tile_pool(name="otf", bufs=1))
        const_pool = ctx.enter_context(tc.tile_pool(name="const", bufs=1))
        w_pool = ctx.enter_context(tc.tile_pool(name="wqkv", bufs=1))
        wo_pool = ctx.enter_context(tc.tile_pool(name="wo", bufs=1))
        x_pool = ctx.enter_context(tc.tile_pool(name="xs", bufs=8))
        cs_pool = ctx.enter_context(tc.tile_pool(name="cs", bufs=1))
        t_pool = ctx.enter_context(tc.tile_pool(name="rott", bufs=2))
        p_pool = ctx.enter_context(tc.tile_pool(name="pexp", bufs=4))
        e_pool = ctx.enter_context(tc.tile_pool(name="epi", bufs=1))
        st_pool = ctx.enter_context(tc.tile_pool(name="stg3", bufs=2))
        # single PSUM pool; tags share the 8 banks across phases:
        #   A: proj accum [128,512] + scores pair [128,1024]   (2x2 banks)
        #   B: rotary pswap out + attn AV accum                (2 banks)
        #   C: rotary bdiag out + out-proj accum               (2 banks)
        ps = ctx.enter_context(tc.tile_pool(name="ps", bufs=1, space="PSUM"))

        qt_s = [qk_pool.tile([128, T], F32, name=f"qt{j}") for j in range(4)]
        kt_s = [qk_pool.tile([128, T], F32, name=f"kt{j}") for j in range(4)]
        v_s = [v_pool.tile([128, NH * 65], BF16, name=f"vt{j}")
               for j in range(NTS)]
        otf = [ot_pool.tile([128, T], BF16, name=f"otf{j}") for j in range(4)]

        # weights first: they gate the first projection matmuls.
        wq_s = [w_pool.tile([128, DH], F32, name=f"wq{k}") for k in range(NKC)]
        wk_s = [w_pool.tile([128, DH], F32, name=f"wk{k}") for k in range(NKC)]
        wv_s = [w_pool.tile([128, DH], F32, name=f"wv{k}") for k in range(NKC)]
        for k in range(NKC):
            ksl = slice(k * 128, (k + 1) * 128)
            nc.sync.dma_start(_r(wq_s[k][:]), _r(wqt[ksl, :]))
        pswap = const_pool.tile([128, 128], F32, name="pswap_s")
        bdiag = const_pool.tile([128, 128], F32, name="bdiag_s")
        trif = const_pool.tile([128, 256], F32, name="trif_s")
        nc.scalar.dma_start(_r(pswap[:]), _r(pswapd[:]))
        nc.scalar.dma_start(_r(bdiag[:]), _r(bdiagd[:]))
        for k in range(NKC):
            ksl = slice(k * 128, (k + 1) * 128)
            nc.scalar.dma_start(_r(wk_s[k][:]), _r(wkt[ksl, :]))
            nc.scalar.dma_start(_r(wv_s[k][:]), _r(wvt[ksl, :]))
        nc.scalar.dma_start(trif[:], trid[:])
        wob = [wo_pool.tile([128, D], BF16, name=f"wob{k}") for k in range(4)]
        for k4 in range(4):
            nc.scalar.dma_start(wob[k4][:], wot[k4 * 128:(k4 + 1) * 128, :])
        epsb = const_pool.tile([128, 1], F32, name="epsb")
        nc.vector.memset(epsb[:], 8.0 * EPS)
        onescb = const_pool.tile([128, NH], BF16, name="onescb")
        nc.vector.memset(onescb[:], 1.0)

        # ---------------- emission helpers ------------------------------
        def emit_x_dma(tt):
            tsl = slice(tt * 512, (tt + 1) * 512)
            xts = []
            for k in range(NKC):
                xc = x_pool.tile([128, 512], F32, name="xc", tag="xc")
                nc.gpsimd.dma_start(_r(xc[:]),
                                    _r(xt[k * 128:(k + 1) * 128, tsl]))
                xts.append(xc)
            return xts

        def py_chain(pib, dt_):
            """Out-projection of one dout chunk of i-block pib (PE filler)."""
            dsl = slice(dt_ * 128, (dt_ + 1) * 128)
            psl = slice(pib * 512, (pib + 1) * 512)
            py = ps.tile([128, 512], F32, name="py", tag="C", bufs=2)
            for k4 in range(4):
                nc.tensor.matmul(py[:], wob[k4][:, dsl], otf[k4][:, psl],
                                 start=(k4 == 0), stop=(k4 == 3))
            st = st_pool.tile([128, 512], BF16, name="st", tag="st", bufs=3)
            nc.vector.tensor_copy(st[:], py[:])
            nc.sync.dma_start(yt[dsl, psl], st[:])

        def proj_closures(tt, xts):
            """12 PE projection k-groups for block tt (q/k/v); ACT copies
            trail. Interleavable with the previous block's attention."""
            tsl = slice(tt * 512, (tt + 1) * 512)
            out = []
            for (wsrc, dsts) in ((wq_s, qt_s), (wk_s, kt_s)):
                for hp in range(4):
                    def g(wsrc=wsrc, dsts=dsts, hp=hp):
                        jsl = slice(hp * 128, (hp + 1) * 128)
                        pp = ps.tile([128, 1024], F32, name="pp", tag="A",
                                     bufs=2)
                        for k in range(NKC):
                            nc.tensor.matmul(
                                pp[:, 0:512], _r(wsrc[k][:, jsl]),
                                _r(xts[k][:]),
                                start=(k == 0), stop=(k == NKC - 1))
                        nc.vector.tensor_copy(_r(dsts[hp][:, tsl]),
                                              pp[:, 0:512])
                    out.append(g)
            for ts_ in range(4):
                def g(ts_=ts_):
                    ci = tt * 4 + ts_
                    pv = ps.tile([128, 1024], F32, name="pv", tag="A", bufs=2)
                    for k in range(NKC):
                        nc.tensor.matmul(
                            pv[:, 0:512],
                            _r(xts[k][:, ts_ * 128:(ts_ + 1) * 128]),
                            _r(wv_s[k][:]),
                            start=(k == 0), stop=(k == NKC - 1))
                    v3 = v_s[ci].rearrange("p (h e) -> p h e", h=NH)
                    nc.vector.tensor_copy(
                        v3[:, :, 0:64],
                        pv[:, 0:512].rearrange("p (h e) -> p h e", h=NH))
                    nc.vector.tensor_copy(v3[:, :, 64:65],
                                          onescb[:].unsqueeze(-1))
                out.append(g)
            return out

        def rot_closures(tt):
            """Rotary + RMS-norm for q and k of block tt. Squares on DVE,
            sqrt on ACT (Square/Sqrt/Copy + Exp/Copy = 2 tables; squares
            and sqrts batch per block so table swaps stay ~2/block)."""
            tsl = slice(tt * 512, (tt + 1) * 512)
            c2t = cs_pool.tile([128, 512], F32, name="c2t", tag="c2t")
            s2t = cs_pool.tile([128, 512], F32, name="s2t", tag="s2t")
            nc.sync.dma_start(c2t[:], c2d[:, tsl])
            nc.sync.dma_start(s2t[:], s2d[:, tsl])
            out = []
            for hp in range(4):
                for nm in ("q", "k"):
                    def g(hp=hp, nm=nm):
                        dst = (qt_s if nm == "q" else kt_s)[hp]
                        sq = t_pool.tile([128, 512], F32, name="sq",
                                         tag=f"sq{nm}")
                        nc.vector.scalar_tensor_tensor(
                            _r(sq[:]), dst[:, tsl], 1.0, dst[:, tsl],
                            mybir.AluOpType.mult, mybir.AluOpType.mult)
                        xs_ = ps.tile([128, 512], F32, name="xs", tag="B",
                                      bufs=2)
                        nc.tensor.matmul(xs_[:], _r(pswap[:]),
                                         _r(dst[:, tsl]),
                                         start=True, stop=True)
                        ms = ps.tile([128, 512], F32, name="ms", tag="C",
                                     bufs=2)
                        nc.tensor.matmul(ms[:], _r(bdiag[:]), _r(sq[:]),
                                         start=True, stop=True)
                        s1 = t_pool.tile([128, 512], F32, name="s1",
                                         tag=f"s1{nm}")
                        nc.scalar.activation(s1[:], ms[:], AF.Sqrt,
                                             scale=0.125, bias=epsb[:])
                        nc.vector.reciprocal_approx_fast(out=s1[:], in_=s1[:])
                        nc.vector.tensor_mul(_r(dst[:, tsl]), dst[:, tsl],
                                             c2t[:])
                        nc.vector.tensor_mul(xs_[:], xs_[:], s2t[:])
                        nc.vector.tensor_add(_r(dst[:, tsl]), dst[:, tsl],
                                             xs_[:])
                        nc.vector.tensor_mul(_r(dst[:, tsl]), dst[:, tsl],
                                             s1[:])
                    out.append(g)
            return out

        def attn_closures(tt):
            """Software-pipelined attention for i-block tt: scores(jt+1) is
            emitted before AV(jt) so interleaved PE work covers exp."""
            ib = tt
            isl = slice(tt * 512, (tt + 1) * 512)
            nj = 4 * ib + 4
            late = tt >= 2

            def emit_sc(hp, jt):
                jsl = slice(jt * 128, (jt + 1) * 128)
                c_ = jt - 4 * ib          # >=0 on diagonal chunks
                off = 128 * c_ if c_ >= 0 else 0
                osc = off if off <= 256 else 256   # keep f32r N>=256
                sc = ps.tile([128, 1024], F32, name="sc", tag="A", bufs=2)
                for h2 in range(2):
                    ho = h2 * 64
                    nc.tensor.matmul(
                        sc[:, 512 * h2 + osc:512 * h2 + 512],
                        _r(kt_s[hp][ho:ho + 64, jsl]),
                        _r(qt_s[hp][ho:ho + 64,
                                    ib * 512 + osc:ib * 512 + 512]),
                        start=True, stop=True)
                return sc, off, c_

            def emit_av(hp, box, jt, p, off):
                for h2 in range(2):
                    h = 2 * hp + h2
                    nc.tensor.matmul(
                        box["ot"][h2][0:65, off:512],
                        v_s[jt][:, 65 * h:65 * h + 65],
                        p[:, 512 * h2 + off:512 * h2 + 512],
                        start=(jt == 0), stop=(jt == nj - 1))

            out = []
            for hp in range(4):
                box = {}

                def c_start(hp=hp, box=box):
                    box["ot"] = [ps.tile([128, 512], F32, name="otp",
                                         tag="B", bufs=2) for _ in range(2)]
                    box["nxt"] = emit_sc(hp, 0)
                    box["avq"] = []
                out.append(c_start)

                for jt in range(nj):
                    def c_item(hp=hp, jt=jt, box=box):
                        sc, off, c_ = box["nxt"]
                        sc3 = sc.rearrange("p (h e) -> p h e", h=2)
                        if c_ >= 0:
                            # additive causal mask (-300 below diag) pre-exp
                            nc.vector.tensor_add(
                                sc3[:, :, off:off + 128],
                                sc3[:, :, off:off + 128],
                                trif[:].rearrange("p (h e) -> p h e", h=2))
                        p = p_pool.tile([128, 1024], BF16, name="p", tag="p")
                        p3 = p.rearrange("p (h e) -> p h e", h=2)
                        nc.scalar.activation(p3[:, :, off:512],
                                             sc3[:, :, off:512], AF.Exp)
                        if jt + 1 < nj:
                            box["nxt"] = emit_sc(hp, jt + 1)
                        # AV runs one pipeline step behind its exp so the
                        # PE never waits on the ACT queue.
                        box["avq"].append((jt, p, off))
                        if len(box["avq"]) > 1:
                            emit_av(hp, box, *box["avq"].pop(0))
                    out.append(c_item)

                def c_epi(hp=hp, box=box):
                    while box["avq"]:
                        emit_av(hp, box, *box["avq"].pop(0))
                    # epilogue: divide by the denominator row (psum row 64).
                    # partition_broadcast is the ONLY gpsimd ucode kernel in
                    # the program, so its IRAM load is paid once.
                    for h2 in range(2):
                        op = box["ot"][h2]
                        den = e_pool.tile([1, 512], F32, name="den",
                                          tag="den")
                        nc.vector.tensor_copy(den[:], op[64:65, :])
                        rden = e_pool.tile([1, 512], F32, name="rden",
                                           tag="rden")
                        nc.vector.reciprocal_approx_fast(out=rden[:],
                                                         in_=den[:])
                        rb = e_pool.tile([64, 512], F32, name="rb", tag="rb")
                        nc.gpsimd.partition_broadcast(rb[:], rden[:],
                                                      channels=64)
                        ho = h2 * 64
                        nc.vector.tensor_mul(otf[hp][ho:ho + 64, isl],
                                             op[0:64, :], rb[:])
                out.append(c_epi)
            return out

        def interleave(a, b):
            ia = ib_ = 0
            while ia < len(a) or ib_ < len(b):
                if ib_ >= len(b) or (ia < len(a)
                                     and ia * len(b) <= ib_ * len(a)):
                    a[ia]()
                    ia += 1
                else:
                    b[ib_]()
                    ib_ += 1

        # ---------------- schedule --------------------------------------
        xts = emit_x_dma(0)
        for f in proj_closures(0, xts):
            f()
        pending_py = []
        for tt in range(NTT):
            a_items = rot_closures(tt) + attn_closures(tt)
            b_items = []
            if tt + 1 < NTT:
                xts = emit_x_dma(tt + 1)
                b_items += proj_closures(tt + 1, xts)
            if tt == NTT - 1:
                take = [p_ for p_ in pending_py if p_[0] <= tt - 1]
            else:
                take = [p_ for p_ in pending_py if p_[0] == tt - 2]
            for p_ in take:
                pending_py.remove(p_)
                b_items.append(lambda p_=p_: py_chain(*p_))
            interleave(a_items, b_items)
            pending_py += [(tt, d) for d in range(8)]
        for pib, d in pending_py:
            py_chain(pib, d)
    return nc


# ---------------- host-side tables & shard prep -------------------------

def host_tables(T: int = 2048):
    n = HD // 4
    af = (1.0 / 1024) ** np.linspace(0, 1, n, dtype=np.float32)
    af = np.concatenate([af, np.zeros(n, np.float32)])  # [32]
    theta = np.outer(np.arange(T, dtype=np.float32), af)  # [T, 32]
    cosT = np.cos(theta).T.astype(np.float32)  # [32, T]
    sinT = np.sin(theta).T.astype(np.float32)
    c2 = np.tile(cosT, (4, 1))                             # [128, T]
    s2 = np.tile(np.concatenate([sinT, -sinT], 0), (2, 1))  # [128, T]
    km = np.arange(128)
    pswap = (km[:, None] == (km[None, :] ^ 32)).astype(np.float32)
    bdiag = ((km[:, None] // 64) == (km[None, :] // 64)).astype(np.float32)
    r_ = np.arange(128)[:, None]
    c_ = np.arange(128)[None, :]
    tri1 = np.where(c_ >= r_, 0.0, -300.0).astype(np.float32)
    tri = np.concatenate([tri1, tri1], axis=1)             # [128, 256]
    return {"c2": np.ascontiguousarray(c2), "s2": np.ascontiguousarray(s2),
            "pswap": pswap, "bdiag": bdiag,
            "trimask": np.ascontiguousarray(tri)}


def _bf16():
    import ml_dtypes
    return ml_dtypes.bfloat16


def core_inputs(x, wq, wk, wv, wo, core: int, T: int = 2048):
    b, g = core % 4, core // 4
    sl = slice(g * DH, (g + 1) * DH)
    m = {
        "xt": np.ascontiguousarray(np.asarray(x[b]).T.astype(np.float32)),
        "wqt": np.ascontiguousarray(np.asarray(wq)[sl, :].T.astype(np.float32)),
        "wkt": np.ascontiguousarray(np.asarray(wk)[sl, :].T.astype(np.float32)),
        "wvt": np.ascontiguousarray(np.asarray(wv)[sl, :].T.astype(np.float32)),
        "wot": np.ascontiguousarray(
            np.asarray(wo)[:, sl].T.astype(_bf16())),
    }
    m.update(host_tables(T))
    return m


_CACHE = {}


def _get_nc(T: int = 2048):
    key = ("nc", T)
    if key not in _CACHE:
        nc = bacc.Bacc("TRN2", target_bir_lowering=False, debug=False)
        build_kernel(nc, T)
        nc.compile()
        _CACHE[key] = nc
    return _CACHE[key]


def kernel(x, wq, wk, wv, wo, mask=None):
    from concourse import bass_utils
    nc = _get_nc(2048)
    in_maps = [core_inputs(x, wq, wk, wv, wo, c) for c in range(8)]
    res = bass_utils.run_bass_kernel_spmd(nc, in_maps, list(range(8)))
    outs = [np.asarray(res.results[c]["yt"]).astype(np.float32)
            for c in range(8)]
    out = np.empty((4, 2048, 1024), np.float32)
    for b in range(4):
        out[b] = (outs[b] + outs[b + 4]).T
    return out
